# revision 1
# baseline (speedup 1.0000x reference)
"""3-layer GCN (gcn_norm message passing) on 8 Trainium2 NeuronCores.

Architecture (v4):
  - Nodes row-sharded across 8 cores (12500 real + 44 pad rows each); per
    layer each core computes h_mm = relu(h_prev) @ W for its shard, scaled by
    dis[src] (norm factorization: norm = dis[dest]*dis[src]), AllGathers the
    bf16 table, then aggregates messages for the destinations it owns.
  - Messages sorted by (dest-group of 4 blocks, source-quarter, dest-block).
    Per (block, quarter) runs are padded to 32-slot units so run boundaries
    are identical on all cores; one dma_gather per (group, quarter) on 4
    parallel SWDGE queues (int16 indices address the bf16 table through 4
    row-windows of 25088 rows).
  - Segment-sum on the TensorEngine: each 128-message chunk contributes one
    N=128 matmul per (statically known) destination block it overlaps, into
    a [128, 512] group PSUM tile; one-hots are 0/1 and built 8 chunks at a
    time with a single wide DVE tensor_tensor (iota_rep == seg broadcast).
  - Group epilogue: out = psum * dis[dest] (DVE wide) + bias (ACT Identity),
    relu (ACT wide), next-layer matmuls (PE), hm = psum2 * dis[own] (ACT).

All data-dependent structure is baked at trace time; the NEFF is compiled
per call and cached in-process.
"""

import os
import sys

sys.path.insert(0, "/opt/trn_rl_repo")

import numpy as np

from concourse import bacc, bass, mybir
from concourse import tile
from concourse import bass_utils

F32 = mybir.dt.float32
BF16 = mybir.dt.bfloat16
I16 = mybir.dt.int16

N_CORES = 8
NQ = 4       # source windows (int16 index range / table rows)
G = 4        # dest blocks per gather group
WOH = 8      # one-hot chunks per wide DVE op
RUN = 32     # run alignment granularity (slots)
PAD_SEG = 10000.0


def _schedule(caps32, ngrp, nblk):
    """Static layout shared by prep and builder.

    caps32: [nblk][NQ] per-(block, quarter) run capacity in RUN-slot units.
    Returns per-call offsets and the chunk->block matmul schedule.
    """
    call_cols = np.zeros(ngrp * NQ + 1, dtype=np.int64)   # gidx col base
    chunk_base = np.zeros(ngrp * NQ + 1, dtype=np.int64)  # chunk id base
    call_nidx = []
    run_slot = {}   # (b, q) -> slot offset of run inside its call
    mm_of_group = []
    for g in range(ngrp):
        blocks = list(range(g * G, min((g + 1) * G, nblk)))
        mms = []
        for q in range(NQ):
            off = 0
            spans = []
            for b in blocks:
                run_slot[(b, q)] = off
                spans.append((b, off, off + caps32[b][q] * RUN))
                off += caps32[b][q] * RUN
            nidx = ((off + 127) // 128) * 128
            call_nidx.append(nidx)
            call_cols[g * NQ + q + 1] = call_cols[g * NQ + q] + nidx // 16
            chunk_base[g * NQ + q + 1] = chunk_base[g * NQ + q] + nidx // 128
            for c in range(nidx // 128):
                lo, hi = c * 128, (c + 1) * 128
                for b, s0, s1 in spans:
                    if s0 < hi and s1 > lo:
                        mms.append((b - g * G, q, c))
        # j-major order for PSUM accumulation bracketing
        mms.sort(key=lambda m: (m[0], m[1], m[2]))
        mm_of_group.append(mms)
    mm_base = np.zeros(ngrp + 1, dtype=np.int64)
    np.cumsum([len(m) for m in mm_of_group], out=mm_base[1:])
    return {
        "call_cols": call_cols, "chunk_base": chunk_base,
        "call_nidx": call_nidx, "run_slot": run_slot,
        "mm_of_group": mm_of_group, "mm_base": mm_base,
        "n_mm": int(mm_base[-1]),
        "gidx_cols": int(call_cols[-1]),
        "total_chunks": int(chunk_base[-1]),
    }


# ----------------------------------------------------------------------------
# Host-side preparation
# ----------------------------------------------------------------------------

def _prep_inputs(x, edge_index, W0, b0, W1, b1, W2, b2, s_real):
    n = x.shape[0]
    assert n % N_CORES == 0 and s_real == n // N_CORES
    nblk = (s_real + 127) // 128
    s_pad = nblk * 128
    total = N_CORES * s_pad
    ngrp = (nblk + G - 1) // G
    assert total % NQ == 0
    wq = total // NQ
    assert wq <= 32767, f"window {wq} exceeds int16 range"

    d = np.asarray(edge_index[0], dtype=np.int64)
    s = np.asarray(edge_index[1], dtype=np.int64)

    deg = np.bincount(s, minlength=n).astype(np.float64) + 1.0
    dis = (1.0 / np.sqrt(deg)).astype(np.float32)

    dests = np.concatenate([d, np.arange(n, dtype=np.int64)])
    srcs = np.concatenate([s, np.arange(n, dtype=np.int64)])

    core = dests // s_real
    dloc = dests - core * s_real
    blk = dloc >> 7
    grp = blk // G
    jj = blk - grp * G
    sg = (srcs // s_real) * s_pad + (srcs % s_real)
    q = sg // wq
    widx = (sg - q * wq).astype(np.int64)

    # per-(core, block, quarter) counts -> shared run capacities (RUN units)
    key = (core * nblk + blk) * NQ + q
    counts = np.bincount(key, minlength=N_CORES * nblk * NQ).reshape(
        N_CORES, nblk, NQ
    )
    caps32 = np.maximum(
        (counts.max(axis=0) + RUN - 1) // RUN, 1
    )  # [nblk, NQ]

    lay = _schedule(caps32.tolist(), ngrp, nblk)

    # rank within (core, b, q)
    order = np.argsort(key, kind="stable")
    inv = np.empty_like(order)
    inv[order] = np.arange(order.size)
    starts = np.zeros(N_CORES * nblk * NQ + 1, dtype=np.int64)
    np.cumsum(counts.reshape(-1), out=starts[1:])
    rank = inv - starts[key]

    run_slot_arr = np.zeros((nblk, NQ), dtype=np.int64)
    for (b, qq), v in lay["run_slot"].items():
        run_slot_arr[b, qq] = v

    slot = run_slot_arr[blk, q] + rank               # slot within call
    call_id = grp * NQ + q
    gcol = lay["call_cols"][call_id] + (slot >> 4)
    grow = slot & 15
    gchunk = lay["chunk_base"][call_id] + (slot >> 7)
    part = slot & 127

    gidx16 = np.zeros((N_CORES, 16, lay["gidx_cols"]), dtype=np.int16)
    gidx16[core, grow, gcol] = widx.astype(np.int16)
    gidx = np.broadcast_to(
        gidx16[:, None, :, :], (N_CORES, 8, 16, lay["gidx_cols"])
    ).reshape(N_CORES, 128, lay["gidx_cols"]).copy()

    # mm col lookup: (gchunk, j) -> column
    mm_col = np.full((lay["total_chunks"], G), -1, dtype=np.int64)
    for g in range(ngrp):
        m0 = lay["mm_base"][g]
        cb = lay["chunk_base"]
        for k, (j, qq, c) in enumerate(lay["mm_of_group"][g]):
            mm_col[cb[g * NQ + qq] + c, j] = m0 + k

    meta = np.full((N_CORES, 128, lay["n_mm"]), PAD_SEG, dtype=np.float32)
    col = mm_col[gchunk, jj]
    assert (col >= 0).all()
    meta[core, part, col] = (dloc - blk * 128).astype(np.float32)

    # dense inputs
    x = np.asarray(x, dtype=np.float32)
    x_t = np.zeros((N_CORES, 128, s_pad), dtype=np.float32)
    dison = np.zeros((N_CORES, 128, nblk), dtype=np.float32)
    disd = np.zeros((N_CORES, 128, s_pad), dtype=np.float32)
    for r in range(N_CORES):
        x_t[r, :, :s_real] = x[r * s_real : (r + 1) * s_real].T
        dv = np.zeros(s_pad, dtype=np.float32)
        dv[:s_real] = dis[r * s_real : (r + 1) * s_real]
        dison[r] = dv.reshape(nblk, 128).T
        disd[r] = dv[None, :]

    wdata = np.zeros((128, 3 * 128 + 3), dtype=np.float32)
    wdata[:, 0:128] = np.asarray(W0, dtype=np.float32)
    wdata[:, 128:256] = np.asarray(W1, dtype=np.float32)
    wdata[:, 256:384] = np.asarray(W2, dtype=np.float32)
    wdata[:, 384] = np.asarray(b0, dtype=np.float32)
    wdata[:, 385] = np.asarray(b1, dtype=np.float32)
    wdata[:, 386] = np.asarray(b2, dtype=np.float32)
    iotar = np.tile(
        np.arange(128, dtype=np.float32), WOH
    )[None, :].repeat(128, axis=0)

    in_maps = [
        {
            "x_t": x_t[r], "meta": meta[r], "gidx": gidx[r],
            "wdata": wdata, "iotar": iotar, "dison": dison[r],
            "disd": disd[r],
        }
        for r in range(N_CORES)
    ]
    sched = {
        "nblk": nblk, "s_pad": s_pad, "s_real": s_real, "ngrp": ngrp,
        "caps32": caps32.tolist(),
    }
    return in_maps, sched


# ----------------------------------------------------------------------------
# Device kernel builder
# ----------------------------------------------------------------------------

def build_kernel(sched, n_cores=N_CORES):
    from contextlib import ExitStack

    nblk, s_pad, ngrp = sched["nblk"], sched["s_pad"], sched["ngrp"]
    caps32 = sched["caps32"]
    lay = _schedule(caps32, ngrp, nblk)
    total = n_cores * s_pad
    wq = total // NQ

    nc = bacc.Bacc(
        "TRN2", target_bir_lowering=False, debug=False, num_devices=n_cores,
        num_swdge_queues=NQ,
    )
    x_t = nc.dram_tensor("x_t", [128, s_pad], F32, kind="ExternalInput")
    meta = nc.dram_tensor("meta", [128, lay["n_mm"]], F32, kind="ExternalInput")
    gidx = nc.dram_tensor("gidx", [128, lay["gidx_cols"]], I16, kind="ExternalInput")
    wdata = nc.dram_tensor("wdata", [128, 3 * 128 + 3], F32, kind="ExternalInput")
    iotar = nc.dram_tensor("iotar", [128, WOH * 128], F32, kind="ExternalInput")
    dison = nc.dram_tensor("dison", [128, nblk], F32, kind="ExternalInput")
    disd = nc.dram_tensor("disd", [128, s_pad], F32, kind="ExternalInput")
    h_out = nc.dram_tensor("h_out", [128, 3 * s_pad], F32, kind="ExternalOutput")

    rg = [list(range(n_cores))]
    ID = mybir.ActivationFunctionType

    with tile.TileContext(nc) as tc, ExitStack() as ctx:
        const = ctx.enter_context(tc.tile_pool(name="const", bufs=1))
        dram = ctx.enter_context(tc.tile_pool(name="dram", bufs=1, space="DRAM"))
        xw = ctx.enter_context(tc.tile_pool(name="xw", bufs=4))
        hmm = ctx.enter_context(tc.tile_pool(name="hmm", bufs=6))
        gath = ctx.enter_context(tc.tile_pool(name="gath", bufs=2 * NQ))
        idxp = ctx.enter_context(tc.tile_pool(name="idxp", bufs=2 * NQ))
        metat = ctx.enter_context(tc.tile_pool(name="metat", bufs=3))
        ohp = ctx.enter_context(tc.tile_pool(name="ohp", bufs=8))
        outsb = ctx.enter_context(tc.tile_pool(name="outsb", bufs=2))
        ddp = ctx.enter_context(tc.tile_pool(name="ddp", bufs=2))
        rsb = ctx.enter_context(tc.tile_pool(name="rsb", bufs=2))
        agg_ps = ctx.enter_context(tc.tile_pool(name="agg_ps", bufs=2, space="PSUM"))
        mm_ps = ctx.enter_context(tc.tile_pool(name="mm_ps", bufs=2, space="PSUM"))
        mma_ps = ctx.enter_context(tc.tile_pool(name="mma_ps", bufs=2, space="PSUM"))

        ag_in = dram.tile([s_pad, 128], BF16)
        ag_outs = [
            dram.tile([total, 128], BF16, addr_space="Shared", name=f"ag_out_l{i}")
            for i in range(3)
        ]

        w_sb = const.tile([128, 3 * 128 + 3], F32)
        nc.sync.dma_start(out=w_sb[:], in_=wdata[:])
        w_bf = const.tile([128, 3 * 128], BF16)
        nc.vector.tensor_copy(w_bf[:], w_sb[:, 0 : 3 * 128])
        iota_sb = const.tile([128, WOH * 128], F32)
        nc.sync.dma_start(out=iota_sb[:], in_=iotar[:])
        dison_sb = const.tile([128, nblk], F32)
        nc.sync.dma_start(out=dison_sb[:], in_=dison[:])

        def bias(L):
            return w_sb[:, 384 + L : 385 + L]

        # ---- Phase A: table0 = (x @ W0) * dis -> ag_in ----
        for b in range(nblk):
            xt = xw.tile([128, 128], F32)
            nc.sync.dma_start(out=xt[:], in_=x_t[:, b * 128 : (b + 1) * 128])
            ps = mma_ps.tile([128, 128], F32, name="psA", tag="psA")
            nc.tensor.matmul(
                ps[:], lhsT=xt[:], rhs=w_sb[:, 0:128], start=True, stop=True
            )
            hm = hmm.tile([128, 128], BF16, name="hmA", tag="hm")
            nc.scalar.activation(
                hm[:], ps[:], ID.Copy, scale=dison_sb[:, b : b + 1]
            )
            nc.scalar.dma_start(out=ag_in[b * 128 : (b + 1) * 128, :], in_=hm[:])

        # ---- 3 layers ----
        for L in range(3):
            ag_out = ag_outs[L]
            nc.gpsimd.collective_compute(
                "AllGather",
                mybir.AluOpType.bypass,
                replica_groups=rg,
                ins=[ag_in[:].opt()],
                outs=[ag_out[:].opt()],
            )
            for g in range(ngrp):
                blocks = list(range(g * G, min((g + 1) * G, nblk)))
                nj = len(blocks)
                gts = []
                for q in range(NQ):
                    nidx = lay["call_nidx"][g * NQ + q]
                    c0 = lay["call_cols"][g * NQ + q]
                    it = idxp.tile([128, nidx // 16], I16, name="it", tag="it")
                    nc.sync.dma_start(
                        out=it[:], in_=gidx[:, c0 : c0 + nidx // 16]
                    )
                    gt = gath.tile([128, nidx], BF16, name="gt", tag="gt")
                    nc.gpsimd.dma_gather(
                        gt[:].rearrange("p (c f) -> p c f", f=128),
                        ag_out[q * wq : (q + 1) * wq, :],
                        it[:],
                        num_idxs=nidx,
                        num_idxs_reg=nidx,
                        elem_size=128,
                        elem_step=128,
                        single_packet=(nidx <= 1024),
                        queue_num=q,
                    )
                    gts.append(gt)

                mms = lay["mm_of_group"][g]
                m0 = int(lay["mm_base"][g])
                n_mm_g = len(mms)
                mt = metat.tile([128, n_mm_g], F32)
                nc.sync.dma_start(out=mt[:], in_=meta[:, m0 : m0 + n_mm_g])

                ohs = {}
                for w0 in range(0, n_mm_g, WOH):
                    wn = min(WOH, n_mm_g - w0)
                    oh = ohp.tile([128, wn * 128], BF16, name="oh", tag="oh")
                    nc.vector.tensor_tensor(
                        oh[:].rearrange("p (c f) -> p c f", f=128),
                        iota_sb[:, : wn * 128].rearrange("p (c f) -> p c f", f=128),
                        mt[:, w0 : w0 + wn].to_broadcast([128, wn, 128]),
                        mybir.AluOpType.is_equal,
                    )
                    ohs[w0] = oh

                ps = agg_ps.tile([128, G * 128], F32, name="aggps", tag="aggps")
                # first/last mm index per block j for start/stop flags
                firsts = {}
                lasts = {}
                for k, (j, qq, c) in enumerate(mms):
                    if j not in firsts:
                        firsts[j] = k
                    lasts[j] = k
                for k, (j, qq, c) in enumerate(mms):
                    cb0 = int(lay["chunk_base"][g * NQ + qq] - lay["chunk_base"][g * NQ])
                    w0 = (k // WOH) * WOH
                    off = k - w0
                    nc.tensor.matmul(
                        ps[:, j * 128 : (j + 1) * 128],
                        lhsT=gts[qq][:, c * 128 : (c + 1) * 128],
                        rhs=ohs[w0][:, off * 128 : (off + 1) * 128],
                        start=(k == firsts[j]),
                        stop=(k == lasts[j]),
                        skip_group_check=True,
                    )
                # ---- group epilogue ----
                w = nj * 128
                gb = g * G * 128
                dd = ddp.tile([128, G * 128], F32, name="dd", tag="dd")
                nc.scalar.dma_start(out=dd[:, :w], in_=disd[:, gb : gb + w])
                ob = outsb.tile([128, G * 128], F32, name="ob", tag="ob")
                nc.vector.tensor_tensor(
                    ob[:, :w], ps[:, :w], dd[:, :w], mybir.AluOpType.mult
                )
                nc.scalar.activation(ob[:, :w], ob[:, :w], ID.Identity, bias=bias(L))
                nc.sync.dma_start(
                    out=h_out[:, L * s_pad + gb : L * s_pad + gb + w],
                    in_=ob[:, :w],
                )
                if L < 2:
                    r = rsb.tile([128, G * 128], BF16, name="r", tag="r")
                    nc.scalar.activation(r[:, :w], ob[:, :w], ID.Relu)
                    ps2 = mm_ps.tile([128, G * 128], F32, name="ps2", tag="ps2")
                    for j in range(nj):
                        nc.tensor.matmul(
                            ps2[:, j * 128 : (j + 1) * 128],
                            lhsT=r[:, j * 128 : (j + 1) * 128],
                            rhs=w_bf[:, (L + 1) * 128 : (L + 2) * 128],
                            start=True,
                            stop=True,
                            skip_group_check=True,
                        )
                    for j in range(nj):
                        b = blocks[j]
                        hm = hmm.tile([128, 128], BF16, name="hm", tag="hm")
                        nc.scalar.activation(
                            hm[:], ps2[:, j * 128 : (j + 1) * 128], ID.Copy,
                            scale=dison_sb[:, b : b + 1],
                        )
                        nc.scalar.dma_start(
                            out=ag_in[b * 128 : (b + 1) * 128, :], in_=hm[:]
                        )

    nc.compile()
    return nc


_BUILD_CACHE = {}


def _get_kernel(sched):
    key = (
        sched["nblk"], sched["s_pad"],
        tuple(tuple(c) for c in sched["caps32"]),
    )
    if key not in _BUILD_CACHE:
        _BUILD_CACHE[key] = build_kernel(sched)
    return _BUILD_CACHE[key]


# ----------------------------------------------------------------------------
# Entry point
# ----------------------------------------------------------------------------

def _run(x, edge_index, W0, b0, W1, b1, W2, b2, trace=False):
    n = int(np.asarray(x).shape[0])
    s_real = n // N_CORES
    in_maps, sched = _prep_inputs(
        x, edge_index, W0, b0, W1, b1, W2, b2, s_real
    )
    s_pad = sched["s_pad"]
    nc = _get_kernel(sched)
    res = bass_utils.run_bass_kernel_spmd(
        nc, in_maps, core_ids=list(range(N_CORES)), trace=trace
    )
    outs = []
    for L in range(3):
        h = np.concatenate(
            [
                res.results[r]["h_out"][:, L * s_pad : L * s_pad + s_real]
                for r in range(N_CORES)
            ],
            axis=1,
        ).T
        outs.append(h)
    full = np.stack(outs, axis=1).astype(np.float32)
    return full, res


def kernel(**inputs):
    trace = os.environ.get("TRN_KERNEL_TRACE", "") == "1"
    out, res = _run(
        np.asarray(inputs["x"]),
        np.asarray(inputs["edge_index"]),
        np.asarray(inputs["W0"]),
        np.asarray(inputs["b0"]),
        np.asarray(inputs["W1"]),
        np.asarray(inputs["b1"]),
        np.asarray(inputs["W2"]),
        np.asarray(inputs["b2"]),
        trace=trace,
    )
    if trace and res.exec_time_ns is not None:
        print(f"HW exec time: {res.exec_time_ns} ns")
        if res.instructions_and_trace:
            print(f"trace: {res.instructions_and_trace[1]}")
    return out



# revision 4
# speedup vs baseline: 1.6521x; 1.6521x over previous
"""3-layer GCN (gcn_norm message passing) on 8 Trainium2 NeuronCores.

Architecture (v5):
  - Nodes row-sharded across 8 cores (12500 real + 44 pad rows each). Per
    layer each core computes h_mm = relu(h_prev) @ W for its shard, scaled by
    dis[src] (norm = dis[dest]*dis[src] factorization), AllGathers the bf16
    table, then aggregates messages for the destinations it owns.
  - The table is laid out in 4 source-quarters, each quarter block-aligned
    within the core shard, so each layer runs FOUR AllGathers (one per
    quarter). A quarter's AG is triggered as soon as the groups producing its
    blocks finish, which overlaps collectives with the previous layer's tail
    and lets gathers start before the full table is assembled.
  - Messages (self-loops excluded) are sorted by (dest-group, source-quarter,
    dest-block, src) with exact per-(block, quarter) run capacities shared
    across cores (max over cores), one dma_gather per (group, quarter) on 4
    SWDGE queues (int16 indices into bf16 quarter windows <= 25600 rows).
  - Segment-sum on the TensorEngine: per dest block an identity matmul first
    adds the self-loop term from the SBUF-resident table, then each 128-slot
    chunk contributes one N=128 matmul per dest block it overlaps (q-major
    order) into a [128, 512] group PSUM tile. One-hots are built 16 chunks at
    a time with wide bf16 DVE tensor_tensor (iota == seg broadcast).
  - Group epilogue: ob = psum * dis[dest] (DVE, bf16 scale), +bias -> bf16
    h_out (ACT), relu (ACT), next-layer matmuls (PE), table write with
    dis[own] scale (ACT) into SBUF tbl + DMA to the quarter AG input.

All data-dependent structure is baked at trace time; the NEFF is compiled
per call and cached in-process.
"""

import os
import sys

sys.path.insert(0, "/opt/trn_rl_repo")

import numpy as np

from concourse import bacc, bass, mybir
from concourse import tile
from concourse import bass_utils

F32 = mybir.dt.float32
BF16 = mybir.dt.bfloat16
I16 = mybir.dt.int16

N_CORES = 8
NQ = 4       # source quarters (int16 windows / AllGather splits)
G = 4        # dest blocks per gather group
WOH = 16     # one-hot chunks per wide DVE tensor_tensor
PAD_SEG = 10000.0
AG_LAG = 2   # groups of slack before emitting a quarter's AG trigger


def _quarters(nblk):
    """Block-aligned source quarters: q -> (first block, n blocks)."""
    base, rem = divmod(nblk, NQ)
    sizes = [base + (1 if i < rem else 0) for i in range(NQ)]
    starts = np.concatenate([[0], np.cumsum(sizes)[:-1]]).astype(np.int64)
    return starts, np.asarray(sizes, dtype=np.int64)


def _schedule(caps, ngrp, nblk):
    """Static layout shared by prep and builder.

    caps: [nblk][NQ] per-(block, quarter) run capacity in slots (exact,
    max over cores). Returns per-call offsets and the chunk->block matmul
    schedule in (quarter, block, chunk) order.
    """
    call_cols = np.zeros(ngrp * NQ + 1, dtype=np.int64)   # gidx col base
    chunk_base = np.zeros(ngrp * NQ + 1, dtype=np.int64)  # chunk id base
    call_nidx = []
    run_slot = {}   # (b, q) -> slot offset of run inside its call
    mm_of_group = []
    for g in range(ngrp):
        blocks = list(range(g * G, min((g + 1) * G, nblk)))
        mms = []
        for q in range(NQ):
            off = 0
            spans = []
            for b in blocks:
                run_slot[(b, q)] = off
                spans.append((b, off, off + caps[b][q]))
                off += caps[b][q]
            nidx = ((off + 127) // 128) * 128
            call_nidx.append(nidx)
            call_cols[g * NQ + q + 1] = call_cols[g * NQ + q] + nidx // 16
            chunk_base[g * NQ + q + 1] = chunk_base[g * NQ + q] + nidx // 128
            for c in range(nidx // 128):
                lo, hi = c * 128, (c + 1) * 128
                for b, s0, s1 in spans:
                    if s0 < hi and s1 > lo:
                        mms.append((b - g * G, q, c))
        # q-major so quarter q's matmuls can start before later quarters land
        mms.sort(key=lambda m: (m[1], m[0], m[2]))
        mm_of_group.append(mms)
    mm_base = np.zeros(ngrp + 1, dtype=np.int64)
    np.cumsum([len(m) for m in mm_of_group], out=mm_base[1:])
    return {
        "call_cols": call_cols, "chunk_base": chunk_base,
        "call_nidx": call_nidx, "run_slot": run_slot,
        "mm_of_group": mm_of_group, "mm_base": mm_base,
        "n_mm": int(mm_base[-1]),
        "gidx_cols": int(call_cols[-1]),
        "total_chunks": int(chunk_base[-1]),
    }


# ----------------------------------------------------------------------------
# Host-side preparation
# ----------------------------------------------------------------------------

def _prep_inputs(x, edge_index, W0, b0, W1, b1, W2, b2, s_real):
    import ml_dtypes

    n = x.shape[0]
    assert n % N_CORES == 0 and s_real == n // N_CORES
    nblk = (s_real + 127) // 128
    s_pad = nblk * 128
    ngrp = (nblk + G - 1) // G
    qstart_b, qsize_b = _quarters(nblk)           # in blocks
    qstart = qstart_b * 128                        # local row starts
    qsize = qsize_b * 128                          # local rows per quarter
    assert all(int(s) * N_CORES <= 32767 for s in qsize)

    d = np.asarray(edge_index[0], dtype=np.int64)
    s = np.asarray(edge_index[1], dtype=np.int64)

    deg = np.bincount(s, minlength=n).astype(np.float64) + 1.0
    dis = (1.0 / np.sqrt(deg)).astype(np.float32)

    # self-loops handled by identity matmuls, not gathered messages
    core = d // s_real
    dloc = d - core * s_real
    blk = dloc >> 7
    grp = blk // G
    jj = blk - grp * G
    score = s // s_real
    sloc = s - score * s_real
    q = np.searchsorted(qstart, sloc, side="right") - 1
    widx = (score * qsize[q] + (sloc - qstart[q])).astype(np.int64)

    # per-(core, block, quarter) counts -> shared exact run capacities
    key = (core * nblk + blk) * NQ + q
    counts = np.bincount(key, minlength=N_CORES * nblk * NQ).reshape(
        N_CORES, nblk, NQ
    )
    caps = np.maximum(counts.max(axis=0), 1)  # [nblk, NQ]

    lay = _schedule(caps.tolist(), ngrp, nblk)

    # rank within (core, b, q), sorted by src for HBM locality
    order = np.lexsort((widx, key))
    inv = np.empty_like(order)
    inv[order] = np.arange(order.size)
    starts = np.zeros(N_CORES * nblk * NQ + 1, dtype=np.int64)
    np.cumsum(counts.reshape(-1), out=starts[1:])
    rank = inv - starts[key]

    run_slot_arr = np.zeros((nblk, NQ), dtype=np.int64)
    for (b, qq), v in lay["run_slot"].items():
        run_slot_arr[b, qq] = v

    slot = run_slot_arr[blk, q] + rank               # slot within call
    call_id = grp * NQ + q
    gcol = lay["call_cols"][call_id] + (slot >> 4)
    grow = slot & 15
    gchunk = lay["chunk_base"][call_id] + (slot >> 7)
    part = slot & 127

    gidx16 = np.zeros((N_CORES, 16, lay["gidx_cols"]), dtype=np.int16)
    gidx16[core, grow, gcol] = widx.astype(np.int16)
    gidx = np.broadcast_to(
        gidx16[:, None, :, :], (N_CORES, 8, 16, lay["gidx_cols"])
    ).reshape(N_CORES, 128, lay["gidx_cols"]).copy()

    # mm col lookup: (gchunk, j) -> column
    mm_col = np.full((lay["total_chunks"], G), -1, dtype=np.int64)
    for g in range(ngrp):
        m0 = lay["mm_base"][g]
        cb = lay["chunk_base"]
        for k, (j, qq, c) in enumerate(lay["mm_of_group"][g]):
            mm_col[cb[g * NQ + qq] + c, j] = m0 + k

    meta = np.full((N_CORES, 128, lay["n_mm"]), PAD_SEG, dtype=np.float32)
    col = mm_col[gchunk, jj]
    assert (col >= 0).all()
    meta[core, part, col] = (dloc - blk * 128).astype(np.float32)
    meta = meta.astype(ml_dtypes.bfloat16)

    # dense inputs
    x = np.asarray(x, dtype=np.float32)
    x_t = np.zeros((N_CORES, 128, s_pad), dtype=np.float32)
    dison = np.zeros((N_CORES, 128, nblk), dtype=np.float32)
    disd = np.zeros((N_CORES, 128, s_pad), dtype=np.float32)
    for r in range(N_CORES):
        x_t[r, :, :s_real] = x[r * s_real : (r + 1) * s_real].T
        dv = np.zeros(s_pad, dtype=np.float32)
        dv[:s_real] = dis[r * s_real : (r + 1) * s_real]
        dison[r] = dv.reshape(nblk, 128).T
        disd[r] = dv[None, :]
    x_t = x_t.astype(ml_dtypes.bfloat16)
    disd = disd.astype(ml_dtypes.bfloat16)

    wdata = np.zeros((128, 3 * 128 + 3), dtype=np.float32)
    wdata[:, 0:128] = np.asarray(W0, dtype=np.float32)
    wdata[:, 128:256] = np.asarray(W1, dtype=np.float32)
    wdata[:, 256:384] = np.asarray(W2, dtype=np.float32)
    wdata[:, 384] = np.asarray(b0, dtype=np.float32)
    wdata[:, 385] = np.asarray(b1, dtype=np.float32)
    wdata[:, 386] = np.asarray(b2, dtype=np.float32)
    iotar = np.tile(
        np.arange(128, dtype=np.float32), WOH
    )[None, :].repeat(128, axis=0).astype(ml_dtypes.bfloat16)
    ident = np.eye(128, dtype=np.float32).astype(ml_dtypes.bfloat16)

    in_maps = [
        {
            "x_t": x_t[r], "meta": meta[r], "gidx": gidx[r],
            "wdata": wdata, "iotar": iotar, "dison": dison[r],
            "disd": disd[r], "ident": ident,
        }
        for r in range(N_CORES)
    ]
    sched = {
        "nblk": nblk, "s_pad": s_pad, "s_real": s_real, "ngrp": ngrp,
        "caps": caps.tolist(),
    }
    return in_maps, sched


# ----------------------------------------------------------------------------
# Device kernel builder
# ----------------------------------------------------------------------------

def build_kernel(sched, n_cores=N_CORES):
    from contextlib import ExitStack

    nblk, s_pad, ngrp = sched["nblk"], sched["s_pad"], sched["ngrp"]
    caps = sched["caps"]
    lay = _schedule(caps, ngrp, nblk)
    qstart_b, qsize_b = _quarters(nblk)
    qstart_b = [int(v) for v in qstart_b]
    qsize_b = [int(v) for v in qsize_b]
    qend_b = [qstart_b[i] + qsize_b[i] - 1 for i in range(NQ)]  # last block
    # group after which quarter q's table blocks are all written
    ag_ready_grp = [qend_b[i] // G for i in range(NQ)]

    nc = bacc.Bacc(
        "TRN2", target_bir_lowering=False, debug=False, num_devices=n_cores,
        num_swdge_queues=NQ,
    )
    x_t = nc.dram_tensor("x_t", [128, s_pad], BF16, kind="ExternalInput")
    meta = nc.dram_tensor("meta", [128, lay["n_mm"]], BF16, kind="ExternalInput")
    gidx = nc.dram_tensor("gidx", [128, lay["gidx_cols"]], I16, kind="ExternalInput")
    wdata = nc.dram_tensor("wdata", [128, 3 * 128 + 3], F32, kind="ExternalInput")
    iotar = nc.dram_tensor("iotar", [128, WOH * 128], BF16, kind="ExternalInput")
    dison = nc.dram_tensor("dison", [128, nblk], F32, kind="ExternalInput")
    disd = nc.dram_tensor("disd", [128, s_pad], BF16, kind="ExternalInput")
    ident_in = nc.dram_tensor("ident", [128, 128], BF16, kind="ExternalInput")
    h_out = nc.dram_tensor("h_out", [128, 3 * s_pad], BF16, kind="ExternalOutput")

    rg = [list(range(n_cores))]
    ID = mybir.ActivationFunctionType

    with tile.TileContext(nc) as tc, ExitStack() as ctx:
        const = ctx.enter_context(tc.tile_pool(name="const", bufs=1))
        dram = ctx.enter_context(tc.tile_pool(name="dram", bufs=1, space="DRAM"))
        tblp = ctx.enter_context(tc.tile_pool(name="tblp", bufs=1))
        xw = ctx.enter_context(tc.tile_pool(name="xw", bufs=2))
        gath = ctx.enter_context(tc.tile_pool(name="gath", bufs=10))
        idxp = ctx.enter_context(tc.tile_pool(name="idxp", bufs=10))
        metat = ctx.enter_context(tc.tile_pool(name="metat", bufs=3))
        ohp = ctx.enter_context(tc.tile_pool(name="ohp", bufs=8))
        outsb = ctx.enter_context(tc.tile_pool(name="outsb", bufs=2))
        obbf = ctx.enter_context(tc.tile_pool(name="obbf", bufs=2))
        rsb = ctx.enter_context(tc.tile_pool(name="rsb", bufs=2))
        agg_ps = ctx.enter_context(tc.tile_pool(name="agg_ps", bufs=2, space="PSUM"))
        mm_ps = ctx.enter_context(tc.tile_pool(name="mm_ps", bufs=2, space="PSUM"))
        mma_ps = ctx.enter_context(tc.tile_pool(name="mma_ps", bufs=2, space="PSUM"))

        ag_ins = [
            dram.tile([qsize_b[q] * 128, 128], BF16, name=f"ag_in_q{q}")
            for q in range(NQ)
        ]
        ag_outs = [
            [
                dram.tile(
                    [n_cores * qsize_b[q] * 128, 128], BF16,
                    addr_space="Shared", name=f"ag_out_l{i}_q{q}",
                )
                for q in range(NQ)
            ]
            for i in range(3)
        ]

        w_sb = const.tile([128, 3 * 128 + 3], F32)
        nc.sync.dma_start(out=w_sb[:], in_=wdata[:])
        w_bf = const.tile([128, 3 * 128], BF16)
        nc.vector.tensor_copy(w_bf[:], w_sb[:, 0 : 3 * 128])
        iota_sb = const.tile([128, WOH * 128], BF16)
        nc.sync.dma_start(out=iota_sb[:], in_=iotar[:])
        dison_sb = const.tile([128, nblk], F32)
        nc.sync.dma_start(out=dison_sb[:], in_=dison[:])
        dd_sb = const.tile([128, s_pad], BF16)
        nc.sync.dma_start(out=dd_sb[:], in_=disd[:])
        ident = const.tile([128, 128], BF16)
        nc.sync.dma_start(out=ident[:], in_=ident_in[:])

        tbl = tblp.tile([128, s_pad], BF16)

        def bias(L):
            return w_sb[:, 384 + L : 385 + L]

        def blk_quarter(b):
            for q in range(NQ):
                if qstart_b[q] <= b <= qend_b[q]:
                    return q
            raise AssertionError(b)

        def write_table_block(b, src_ps):
            """ACT: tbl[:, b] = src_ps * dis[own block b]; DMA to AG input."""
            tslice = tbl[:, b * 128 : (b + 1) * 128]
            nc.scalar.activation(
                tslice, src_ps, ID.Copy, scale=dison_sb[:, b : b + 1]
            )
            q = blk_quarter(b)
            r0 = (b - qstart_b[q]) * 128
            nc.scalar.dma_start(
                out=ag_ins[q][r0 : r0 + 128, :], in_=tslice
            )

        def trigger_ag(L, q):
            nc.gpsimd.collective_compute(
                "AllGather",
                mybir.AluOpType.bypass,
                replica_groups=rg,
                ins=[ag_ins[q][:].opt()],
                outs=[ag_outs[L][q][:].opt()],
            )

        # ---- Phase A: table0 = (x @ W0) * dis -> tbl + ag_in ----
        XW_BLKS = 16
        for b in range(nblk):
            if b % XW_BLKS == 0:
                xt = xw.tile([128, XW_BLKS * 128], BF16, name="xt", tag="xt")
                w = min(XW_BLKS * 128, s_pad - b * 128)
                nc.sync.dma_start(
                    out=xt[:, :w], in_=x_t[:, b * 128 : b * 128 + w]
                )
            o = (b % XW_BLKS) * 128
            ps = mma_ps.tile([128, 128], F32, name="psA", tag="psA")
            nc.tensor.matmul(
                ps[:], lhsT=xt[:, o : o + 128], rhs=w_bf[:, 0:128],
                start=True, stop=True,
            )
            write_table_block(b, ps[:])
            for q in range(NQ):
                # lag a couple of blocks so the AG trigger's deps are done
                if b == min(qend_b[q] + 2, nblk - 1):
                    trigger_ag(0, q)

        # ---- 3 layers ----
        for L in range(3):
            for g in range(ngrp):
                blocks = list(range(g * G, min((g + 1) * G, nblk)))
                nj = len(blocks)
                gts = []
                for q in range(NQ):
                    nidx = lay["call_nidx"][g * NQ + q]
                    c0 = lay["call_cols"][g * NQ + q]
                    it = idxp.tile([128, nidx // 16], I16, name="it", tag="it")
                    nc.sync.dma_start(
                        out=it[:], in_=gidx[:, c0 : c0 + nidx // 16]
                    )
                    gt = gath.tile([128, nidx], BF16, name="gt", tag="gt")
                    nc.gpsimd.dma_gather(
                        gt[:].rearrange("p (c f) -> p c f", f=128),
                        ag_outs[L][q][:],
                        it[:],
                        num_idxs=nidx,
                        num_idxs_reg=nidx,
                        elem_size=128,
                        elem_step=128,
                        single_packet=(nidx <= 1024),
                        queue_num=q,
                    )
                    gts.append(gt)

                mms = lay["mm_of_group"][g]
                m0 = int(lay["mm_base"][g])
                n_mm_g = len(mms)
                mt = metat.tile([128, n_mm_g], BF16)
                nc.sync.dma_start(out=mt[:], in_=meta[:, m0 : m0 + n_mm_g])

                ohs = {}
                for w0 in range(0, n_mm_g, WOH):
                    wn = min(WOH, n_mm_g - w0)
                    oh = ohp.tile([128, wn * 128], BF16, name="oh", tag="oh")
                    nc.vector.tensor_tensor(
                        oh[:].rearrange("p (c f) -> p c f", f=128),
                        iota_sb[:, : wn * 128].rearrange("p (c f) -> p c f", f=128),
                        mt[:, w0 : w0 + wn].to_broadcast([128, wn, 128]),
                        mybir.AluOpType.is_equal,
                    )
                    ohs[w0] = oh

                ps = agg_ps.tile([128, G * 128], F32, name="aggps", tag="aggps")
                # ONE start per PSUM bank (start=True clears has_written for
                # the whole bank): only the very first matmul starts, only the
                # very last stops; each element's first writer overwrites.
                for j, b in enumerate(blocks):
                    nc.tensor.matmul(
                        ps[:, j * 128 : (j + 1) * 128],
                        lhsT=tbl[:, b * 128 : (b + 1) * 128],
                        rhs=ident[:],
                        start=(j == 0),
                        stop=False,
                        skip_group_check=True,
                    )
                for k, (j, qq, c) in enumerate(mms):
                    w0 = (k // WOH) * WOH
                    off = k - w0
                    nc.tensor.matmul(
                        ps[:, j * 128 : (j + 1) * 128],
                        lhsT=gts[qq][:, c * 128 : (c + 1) * 128],
                        rhs=ohs[w0][:, off * 128 : (off + 1) * 128],
                        start=False,
                        stop=(k == n_mm_g - 1),
                        skip_group_check=True,
                    )
                # ---- group epilogue ----
                w = nj * 128
                gb = g * G * 128
                ob = outsb.tile([128, G * 128], F32, name="ob", tag="ob")
                nc.vector.tensor_tensor(
                    ob[:, :w], ps[:, :w], dd_sb[:, gb : gb + w],
                    mybir.AluOpType.mult,
                )
                obf = obbf.tile([128, G * 128], BF16, name="obf", tag="obf")
                nc.scalar.activation(
                    obf[:, :w], ob[:, :w], ID.Identity, bias=bias(L)
                )
                nc.sync.dma_start(
                    out=h_out[:, L * s_pad + gb : L * s_pad + gb + w],
                    in_=obf[:, :w],
                )
                if L < 2:
                    r = rsb.tile([128, G * 128], BF16, name="r", tag="r")
                    nc.scalar.activation(r[:, :w], ob[:, :w], ID.Relu, bias=bias(L))
                    ps2 = mm_ps.tile([128, G * 128], F32, name="ps2", tag="ps2")
                    for j in range(nj):
                        nc.tensor.matmul(
                            ps2[:, j * 128 : (j + 1) * 128],
                            lhsT=r[:, j * 128 : (j + 1) * 128],
                            rhs=w_bf[:, (L + 1) * 128 : (L + 2) * 128],
                            start=True,
                            stop=True,
                            skip_group_check=True,
                        )
                    for j, b in enumerate(blocks):
                        write_table_block(b, ps2[:, j * 128 : (j + 1) * 128])
                    for q in range(NQ):
                        if g == min(ag_ready_grp[q] + AG_LAG, ngrp - 1):
                            trigger_ag(L + 1, q)

    nc.compile()
    return nc


_BUILD_CACHE = {}


def _get_kernel(sched):
    key = (
        sched["nblk"], sched["s_pad"],
        tuple(tuple(c) for c in sched["caps"]),
    )
    if key not in _BUILD_CACHE:
        _BUILD_CACHE[key] = build_kernel(sched)
    return _BUILD_CACHE[key]


# ----------------------------------------------------------------------------
# Entry point
# ----------------------------------------------------------------------------

def _run(x, edge_index, W0, b0, W1, b1, W2, b2, trace=False):
    n = int(np.asarray(x).shape[0])
    s_real = n // N_CORES
    in_maps, sched = _prep_inputs(
        x, edge_index, W0, b0, W1, b1, W2, b2, s_real
    )
    s_pad = sched["s_pad"]
    nc = _get_kernel(sched)
    res = bass_utils.run_bass_kernel_spmd(
        nc, in_maps, core_ids=list(range(N_CORES)), trace=trace
    )
    outs = []
    for L in range(3):
        h = np.concatenate(
            [
                np.asarray(
                    res.results[r]["h_out"][:, L * s_pad : L * s_pad + s_real],
                    dtype=np.float32,
                )
                for r in range(N_CORES)
            ],
            axis=1,
        ).T
        outs.append(h)
    full = np.stack(outs, axis=1).astype(np.float32)
    return full, res


def kernel(**inputs):
    trace = os.environ.get("TRN_KERNEL_TRACE", "") == "1"
    out, res = _run(
        np.asarray(inputs["x"]),
        np.asarray(inputs["edge_index"]),
        np.asarray(inputs["W0"]),
        np.asarray(inputs["b0"]),
        np.asarray(inputs["W1"]),
        np.asarray(inputs["b1"]),
        np.asarray(inputs["W2"]),
        np.asarray(inputs["b2"]),
        trace=trace,
    )
    if trace and res.exec_time_ns is not None:
        print(f"HW exec time: {res.exec_time_ns} ns")
        if res.instructions_and_trace:
            print(f"trace: {res.instructions_and_trace[1]}")
    return out


# revision 12
# speedup vs baseline: 1.6960x; 1.0266x over previous
"""3-layer GCN (gcn_norm message passing) on 8 Trainium2 NeuronCores.

Architecture (v5):
  - Nodes row-sharded across 8 cores (12500 real + 44 pad rows each). Per
    layer each core computes h_mm = relu(h_prev) @ W for its shard, scaled by
    dis[src] (norm = dis[dest]*dis[src] factorization), AllGathers the bf16
    table, then aggregates messages for the destinations it owns.
  - The table is laid out in 4 source-quarters, each quarter block-aligned
    within the core shard, so each layer runs FOUR AllGathers (one per
    quarter). A quarter's AG is triggered as soon as the groups producing its
    blocks finish, which overlaps collectives with the previous layer's tail
    and lets gathers start before the full table is assembled.
  - Messages (self-loops excluded) are sorted by (dest-group, source-quarter,
    dest-block, src) with exact per-(block, quarter) run capacities shared
    across cores (max over cores), one dma_gather per (group, quarter) on 4
    SWDGE queues (int16 indices into bf16 quarter windows <= 25600 rows).
  - Segment-sum on the TensorEngine: per dest block an identity matmul first
    adds the self-loop term from the SBUF-resident table, then each 128-slot
    chunk contributes one N=128 matmul per dest block it overlaps (q-major
    order) into a [128, 512] group PSUM tile. One-hots are built 16 chunks at
    a time with wide bf16 DVE tensor_tensor (iota == seg broadcast).
  - Group epilogue: ob = psum * dis[dest] (DVE, bf16 scale), +bias -> bf16
    h_out (ACT), relu (ACT), next-layer matmuls (PE), table write with
    dis[own] scale (ACT) into SBUF tbl + DMA to the quarter AG input.

All data-dependent structure is baked at trace time; the NEFF is compiled
per call and cached in-process.
"""

import os
import sys

sys.path.insert(0, "/opt/trn_rl_repo")

import numpy as np

from concourse import bacc, bass, mybir
from concourse import tile
from concourse import bass_utils

F32 = mybir.dt.float32
BF16 = mybir.dt.bfloat16
I16 = mybir.dt.int16

N_CORES = 8
NQ = 4       # source quarters (int16 windows / AllGather splits)
G = 4        # dest blocks per gather group
WOH = 16     # one-hot chunks per wide DVE tensor_tensor
PAD_SEG = 10000.0
AG_LAG = 2   # groups of slack before emitting a quarter's AG trigger


def _quarters(nblk):
    """Block-aligned source quarters: q -> (first block, n blocks)."""
    base, rem = divmod(nblk, NQ)
    sizes = [base + (1 if i < rem else 0) for i in range(NQ)]
    starts = np.concatenate([[0], np.cumsum(sizes)[:-1]]).astype(np.int64)
    return starts, np.asarray(sizes, dtype=np.int64)


def _schedule(caps, ngrp, nblk):
    """Static layout shared by prep and builder.

    caps: [nblk][NQ] per-(block, quarter) run capacity in slots (exact,
    max over cores). Returns per-call offsets and the chunk->block matmul
    schedule in (quarter, block, chunk) order.
    """
    call_cols = np.zeros(ngrp * NQ + 1, dtype=np.int64)   # gidx col base
    chunk_base = np.zeros(ngrp * NQ + 1, dtype=np.int64)  # chunk id base
    call_nidx = []
    run_slot = {}   # (b, q) -> slot offset of run inside its call
    mm_of_group = []
    for g in range(ngrp):
        blocks = list(range(g * G, min((g + 1) * G, nblk)))
        mms = []
        for q in range(NQ):
            off = 0
            spans = []
            for b in blocks:
                run_slot[(b, q)] = off
                spans.append((b, off, off + caps[b][q]))
                off += caps[b][q]
            nidx = ((off + 127) // 128) * 128
            call_nidx.append(nidx)
            call_cols[g * NQ + q + 1] = call_cols[g * NQ + q] + nidx // 16
            chunk_base[g * NQ + q + 1] = chunk_base[g * NQ + q] + nidx // 128
            for c in range(nidx // 128):
                lo, hi = c * 128, (c + 1) * 128
                for b, s0, s1 in spans:
                    if s0 < hi and s1 > lo:
                        mms.append((b - g * G, q, c))
        # q-major so quarter q's matmuls can start before later quarters land
        mms.sort(key=lambda m: (m[1], m[0], m[2]))
        mm_of_group.append(mms)
    mm_base = np.zeros(ngrp + 1, dtype=np.int64)
    np.cumsum([len(m) for m in mm_of_group], out=mm_base[1:])
    return {
        "call_cols": call_cols, "chunk_base": chunk_base,
        "call_nidx": call_nidx, "run_slot": run_slot,
        "mm_of_group": mm_of_group, "mm_base": mm_base,
        "n_mm": int(mm_base[-1]),
        "gidx_cols": int(call_cols[-1]),
        "total_chunks": int(chunk_base[-1]),
    }


# ----------------------------------------------------------------------------
# Host-side preparation
# ----------------------------------------------------------------------------

def _prep_inputs(x, edge_index, W0, b0, W1, b1, W2, b2, s_real):
    import ml_dtypes

    n = x.shape[0]
    assert n % N_CORES == 0 and s_real == n // N_CORES
    nblk = (s_real + 127) // 128
    s_pad = nblk * 128
    ngrp = (nblk + G - 1) // G
    qstart_b, qsize_b = _quarters(nblk)           # in blocks
    qstart = qstart_b * 128                        # local row starts
    qsize = qsize_b * 128                          # local rows per quarter
    assert all(int(s) * N_CORES <= 32767 for s in qsize)

    d = np.asarray(edge_index[0], dtype=np.int64)
    s = np.asarray(edge_index[1], dtype=np.int64)

    deg = np.bincount(s, minlength=n).astype(np.float64) + 1.0
    dis = (1.0 / np.sqrt(deg)).astype(np.float32)

    # self-loops handled by identity matmuls, not gathered messages
    core = d // s_real
    dloc = d - core * s_real
    blk = dloc >> 7
    grp = blk // G
    jj = blk - grp * G
    score = s // s_real
    sloc = s - score * s_real
    q = np.searchsorted(qstart, sloc, side="right") - 1
    widx = (score * qsize[q] + (sloc - qstart[q])).astype(np.int64)

    # per-(core, block, quarter) counts -> shared exact run capacities
    key = (core * nblk + blk) * NQ + q
    counts = np.bincount(key, minlength=N_CORES * nblk * NQ).reshape(
        N_CORES, nblk, NQ
    )
    caps = np.maximum(counts.max(axis=0), 1)  # [nblk, NQ]

    lay = _schedule(caps.tolist(), ngrp, nblk)

    # rank within (core, b, q), sorted by src for HBM locality
    order = np.lexsort((widx, key))
    inv = np.empty_like(order)
    inv[order] = np.arange(order.size)
    starts = np.zeros(N_CORES * nblk * NQ + 1, dtype=np.int64)
    np.cumsum(counts.reshape(-1), out=starts[1:])
    rank = inv - starts[key]

    run_slot_arr = np.zeros((nblk, NQ), dtype=np.int64)
    for (b, qq), v in lay["run_slot"].items():
        run_slot_arr[b, qq] = v

    slot = run_slot_arr[blk, q] + rank               # slot within call
    call_id = grp * NQ + q
    gcol = lay["call_cols"][call_id] + (slot >> 4)
    grow = slot & 15
    gchunk = lay["chunk_base"][call_id] + (slot >> 7)
    part = slot & 127

    gidx16 = np.zeros((N_CORES, 16, lay["gidx_cols"]), dtype=np.int16)
    gidx16[core, grow, gcol] = widx.astype(np.int16)
    gidx = np.broadcast_to(
        gidx16[:, None, :, :], (N_CORES, 8, 16, lay["gidx_cols"])
    ).reshape(N_CORES, 128, lay["gidx_cols"]).copy()

    # mm col lookup: (gchunk, j) -> column
    mm_col = np.full((lay["total_chunks"], G), -1, dtype=np.int64)
    for g in range(ngrp):
        m0 = lay["mm_base"][g]
        cb = lay["chunk_base"]
        for k, (j, qq, c) in enumerate(lay["mm_of_group"][g]):
            mm_col[cb[g * NQ + qq] + c, j] = m0 + k

    meta = np.full((N_CORES, 128, lay["n_mm"]), PAD_SEG, dtype=np.float32)
    col = mm_col[gchunk, jj]
    assert (col >= 0).all()
    meta[core, part, col] = (dloc - blk * 128).astype(np.float32)
    meta = meta.astype(ml_dtypes.bfloat16)

    # dense inputs
    x = np.asarray(x, dtype=np.float32)
    x_t = np.zeros((N_CORES, 128, s_pad), dtype=np.float32)
    dison = np.zeros((N_CORES, 128, nblk), dtype=np.float32)
    disd = np.zeros((N_CORES, 128, s_pad), dtype=np.float32)
    for r in range(N_CORES):
        x_t[r, :, :s_real] = x[r * s_real : (r + 1) * s_real].T
        dv = np.zeros(s_pad, dtype=np.float32)
        dv[:s_real] = dis[r * s_real : (r + 1) * s_real]
        dison[r] = dv.reshape(nblk, 128).T
        disd[r] = dv[None, :]
    x_t = x_t.astype(ml_dtypes.bfloat16)
    disd = disd.astype(ml_dtypes.bfloat16)

    wdata = np.zeros((128, 3 * 128 + 3), dtype=np.float32)
    wdata[:, 0:128] = np.asarray(W0, dtype=np.float32)
    wdata[:, 128:256] = np.asarray(W1, dtype=np.float32)
    wdata[:, 256:384] = np.asarray(W2, dtype=np.float32)
    wdata[:, 384] = np.asarray(b0, dtype=np.float32)
    wdata[:, 385] = np.asarray(b1, dtype=np.float32)
    wdata[:, 386] = np.asarray(b2, dtype=np.float32)
    iotar = np.tile(
        np.arange(128, dtype=np.float32), WOH
    )[None, :].repeat(128, axis=0).astype(ml_dtypes.bfloat16)
    ident = np.eye(128, dtype=np.float32).astype(ml_dtypes.bfloat16)

    in_maps = [
        {
            "x_t": x_t[r], "meta": meta[r], "gidx": gidx[r],
            "wdata": wdata, "iotar": iotar, "dison": dison[r],
            "disd": disd[r], "ident": ident,
        }
        for r in range(N_CORES)
    ]
    sched = {
        "nblk": nblk, "s_pad": s_pad, "s_real": s_real, "ngrp": ngrp,
        "caps": caps.tolist(),
    }
    return in_maps, sched


# ----------------------------------------------------------------------------
# Device kernel builder
# ----------------------------------------------------------------------------

def build_kernel(sched, n_cores=N_CORES):
    from contextlib import ExitStack

    nblk, s_pad, ngrp = sched["nblk"], sched["s_pad"], sched["ngrp"]
    caps = sched["caps"]
    lay = _schedule(caps, ngrp, nblk)
    qstart_b, qsize_b = _quarters(nblk)
    qstart_b = [int(v) for v in qstart_b]
    qsize_b = [int(v) for v in qsize_b]
    qend_b = [qstart_b[i] + qsize_b[i] - 1 for i in range(NQ)]  # last block
    # group after which quarter q's table blocks are all written
    ag_ready_grp = [qend_b[i] // G for i in range(NQ)]

    nc = bacc.Bacc(
        "TRN2", target_bir_lowering=False, debug=False, num_devices=n_cores,
        num_swdge_queues=NQ,
    )
    x_t = nc.dram_tensor("x_t", [128, s_pad], BF16, kind="ExternalInput")
    meta = nc.dram_tensor("meta", [128, lay["n_mm"]], BF16, kind="ExternalInput")
    gidx = nc.dram_tensor("gidx", [128, lay["gidx_cols"]], I16, kind="ExternalInput")
    wdata = nc.dram_tensor("wdata", [128, 3 * 128 + 3], F32, kind="ExternalInput")
    iotar = nc.dram_tensor("iotar", [128, WOH * 128], BF16, kind="ExternalInput")
    dison = nc.dram_tensor("dison", [128, nblk], F32, kind="ExternalInput")
    disd = nc.dram_tensor("disd", [128, s_pad], BF16, kind="ExternalInput")
    ident_in = nc.dram_tensor("ident", [128, 128], BF16, kind="ExternalInput")
    h_out = nc.dram_tensor("h_out", [128, 3 * s_pad], BF16, kind="ExternalOutput")

    rg = [list(range(n_cores))]
    ID = mybir.ActivationFunctionType

    with tile.TileContext(nc) as tc, ExitStack() as ctx:
        const = ctx.enter_context(tc.tile_pool(name="const", bufs=1))
        dram = ctx.enter_context(tc.tile_pool(name="dram", bufs=1, space="DRAM"))
        tblp = ctx.enter_context(tc.tile_pool(name="tblp", bufs=1))
        xw = ctx.enter_context(tc.tile_pool(name="xw", bufs=2))
        gath = ctx.enter_context(tc.tile_pool(name="gath", bufs=12))
        idxp = ctx.enter_context(tc.tile_pool(name="idxp", bufs=12))
        metat = ctx.enter_context(tc.tile_pool(name="metat", bufs=4))
        ohp = ctx.enter_context(tc.tile_pool(name="ohp", bufs=8))
        outsb = ctx.enter_context(tc.tile_pool(name="outsb", bufs=3))
        obbf = ctx.enter_context(tc.tile_pool(name="obbf", bufs=3))
        rsb = ctx.enter_context(tc.tile_pool(name="rsb", bufs=3))
        agg_ps = ctx.enter_context(tc.tile_pool(name="agg_ps", bufs=3, space="PSUM"))
        mm_ps = ctx.enter_context(tc.tile_pool(name="mm_ps", bufs=2, space="PSUM"))
        mma_ps = ctx.enter_context(tc.tile_pool(name="mma_ps", bufs=2, space="PSUM"))

        ag_ins = [
            dram.tile([qsize_b[q] * 128, 128], BF16, name=f"ag_in_q{q}")
            for q in range(NQ)
        ]
        ag_outs = [
            [
                dram.tile(
                    [n_cores * qsize_b[q] * 128, 128], BF16,
                    addr_space="Shared", name=f"ag_out_l{i}_q{q}",
                )
                for q in range(NQ)
            ]
            for i in range(3)
        ]

        w_sb = const.tile([128, 3 * 128 + 3], F32)
        nc.sync.dma_start(out=w_sb[:], in_=wdata[:])
        w_bf = const.tile([128, 3 * 128], BF16)
        nc.vector.tensor_copy(w_bf[:], w_sb[:, 0 : 3 * 128])
        iota_sb = const.tile([128, WOH * 128], BF16)
        nc.sync.dma_start(out=iota_sb[:], in_=iotar[:])
        dison_sb = const.tile([128, nblk], F32)
        nc.sync.dma_start(out=dison_sb[:], in_=dison[:])
        dd_sb = const.tile([128, s_pad], BF16)
        nc.sync.dma_start(out=dd_sb[:], in_=disd[:])
        ident = const.tile([128, 128], BF16)
        nc.sync.dma_start(out=ident[:], in_=ident_in[:])

        tbl = tblp.tile([128, s_pad], BF16)

        def bias(L):
            return w_sb[:, 384 + L : 385 + L]

        def blk_quarter(b):
            for q in range(NQ):
                if qstart_b[q] <= b <= qend_b[q]:
                    return q
            raise AssertionError(b)

        def scale_table_block(b, src_ps):
            """ACT: tbl[:, b] = src_ps * dis[own block b]."""
            nc.scalar.activation(
                tbl[:, b * 128 : (b + 1) * 128], src_ps, ID.Copy,
                scale=dison_sb[:, b : b + 1],
            )

        def flush_table_blocks(b0, b1):
            """DMA tbl blocks [b0, b1) to AG inputs, split on quarter edges.

            The DRAM rows interleave blocks (node = block*128 + partition), so
            both sides use matching [partition, block, feat] access patterns.
            """
            b = b0
            while b < b1:
                q = blk_quarter(b)
                e = min(b1, qend_b[q] + 1)
                r0 = (b - qstart_b[q]) * 128
                n = e - b
                nc.scalar.dma_start(
                    out=ag_ins[q][r0 : r0 + n * 128, :].rearrange(
                        "(k i) f -> i k f", i=128
                    ),
                    in_=tbl[:, b * 128 : e * 128].rearrange(
                        "p (k f) -> p k f", f=128
                    ),
                )
                b = e

        ag_done = set()

        def trigger_ag(L, q):
            if (L, q) in ag_done:
                return
            ag_done.add((L, q))
            nc.gpsimd.collective_compute(
                "AllGather",
                mybir.AluOpType.bypass,
                replica_groups=rg,
                ins=[ag_ins[q][:].opt()],
                outs=[ag_outs[L][q][:].opt()],
            )

        # warmup collective: absorbs the first-collective barrier/setup cost
        # concurrently with Phase A instead of delaying the first real AG
        warm_in = dram.tile([128, 128], BF16, name="warm_in")
        warm_out = dram.tile([n_cores * 128, 128], BF16,
                             addr_space="Shared", name="warm_out")
        nc.sync.dma_start(out=warm_in[:], in_=ident_in[:])
        nc.gpsimd.collective_compute(
            "AllGather",
            mybir.AluOpType.bypass,
            replica_groups=rg,
            ins=[warm_in[:].opt()],
            outs=[warm_out[:].opt()],
        )

        # ---- Phase A: table0 = (x @ W0) * dis -> tbl + ag_in ----
        XW_BLKS = 16
        FLUSH = 8
        flushed = 0
        for b in range(nblk):
            if b % XW_BLKS == 0:
                xt = xw.tile([128, XW_BLKS * 128], BF16, name="xt", tag="xt")
                w = min(XW_BLKS * 128, s_pad - b * 128)
                nc.sync.dma_start(
                    out=xt[:, :w], in_=x_t[:, b * 128 : b * 128 + w]
                )
            o = (b % XW_BLKS) * 128
            ps = mma_ps.tile([128, 128], F32, name="psA", tag="psA")
            nc.tensor.matmul(
                ps[:], lhsT=xt[:, o : o + 128], rhs=w_bf[:, 0:128],
                start=True, stop=True,
            )
            scale_table_block(b, ps[:])
            if b + 1 - flushed >= FLUSH or b == nblk - 1 or b in qend_b:
                flush_table_blocks(flushed, b + 1)
                flushed = b + 1
            for q in range(NQ):
                # lag a couple of blocks so the AG trigger's deps are done
                if b == min(qend_b[q] + 2, nblk - 1):
                    trigger_ag(0, q)

        # ---- 3 layers, software-pipelined emission: the aggregation of
        # group g+1 is emitted before the epilogue of group g so neither the
        # DVE (one-hots vs psum-read) nor the PE (chunk mms vs ps2 mms)
        # head-of-line blocks on the other group's dependencies.
        def agg_phase(L, g):
            blocks = list(range(g * G, min((g + 1) * G, nblk)))
            gts = []
            for q in range(NQ):
                trigger_ag(L, q)  # no-op unless not yet emitted (lazy)
                nidx = lay["call_nidx"][g * NQ + q]
                c0 = lay["call_cols"][g * NQ + q]
                it = idxp.tile([128, nidx // 16], I16, name="it", tag="it")
                nc.sync.dma_start(
                    out=it[:], in_=gidx[:, c0 : c0 + nidx // 16]
                )
                gt = gath.tile([128, nidx], BF16, name="gt", tag="gt")
                nc.gpsimd.dma_gather(
                    gt[:].rearrange("p (c f) -> p c f", f=128),
                    ag_outs[L][q][:],
                    it[:],
                    num_idxs=nidx,
                    num_idxs_reg=nidx,
                    elem_size=128,
                    elem_step=128,
                    single_packet=(nidx <= 1024),
                    queue_num=q,
                )
                gts.append(gt)

            mms = lay["mm_of_group"][g]
            m0 = int(lay["mm_base"][g])
            n_mm_g = len(mms)
            mt = metat.tile([128, n_mm_g], BF16)
            nc.sync.dma_start(out=mt[:], in_=meta[:, m0 : m0 + n_mm_g])

            ohs = {}
            for w0 in range(0, n_mm_g, WOH):
                wn = min(WOH, n_mm_g - w0)
                oh = ohp.tile([128, wn * 128], BF16, name="oh", tag="oh")
                nc.vector.tensor_tensor(
                    oh[:].rearrange("p (c f) -> p c f", f=128),
                    iota_sb[:, : wn * 128].rearrange("p (c f) -> p c f", f=128),
                    mt[:, w0 : w0 + wn].to_broadcast([128, wn, 128]),
                    mybir.AluOpType.is_equal,
                )
                ohs[w0] = oh

            ps = agg_ps.tile([128, G * 128], F32, name="aggps", tag="aggps")
            # ONE start per PSUM bank (start=True clears has_written for
            # the whole bank): only the very first matmul starts, only the
            # very last stops; each element's first writer overwrites.
            for j, b in enumerate(blocks):
                nc.tensor.matmul(
                    ps[:, j * 128 : (j + 1) * 128],
                    lhsT=tbl[:, b * 128 : (b + 1) * 128],
                    rhs=ident[:],
                    start=(j == 0),
                    stop=False,
                    skip_group_check=True,
                )
            for k, (j, qq, c) in enumerate(mms):
                w0 = (k // WOH) * WOH
                off = k - w0
                nc.tensor.matmul(
                    ps[:, j * 128 : (j + 1) * 128],
                    lhsT=gts[qq][:, c * 128 : (c + 1) * 128],
                    rhs=ohs[w0][:, off * 128 : (off + 1) * 128],
                    start=False,
                    stop=(k == n_mm_g - 1),
                    skip_group_check=True,
                )
            return (L, g, blocks, ps)

        def epi_phase(st):
            L, g, blocks, ps = st
            nj = len(blocks)
            w = nj * 128
            gb = g * G * 128
            ob = outsb.tile([128, G * 128], F32, name="ob", tag="ob")
            nc.vector.tensor_tensor(
                ob[:, :w], ps[:, :w], dd_sb[:, gb : gb + w],
                mybir.AluOpType.mult,
            )
            obf = obbf.tile([128, G * 128], BF16, name="obf", tag="obf")
            nc.scalar.activation(
                obf[:, :w], ob[:, :w], ID.Identity, bias=bias(L)
            )
            nc.sync.dma_start(
                out=h_out[:, L * s_pad + gb : L * s_pad + gb + w],
                in_=obf[:, :w],
            )
            if L < 2:
                r = rsb.tile([128, G * 128], BF16, name="r", tag="r")
                nc.scalar.activation(r[:, :w], ob[:, :w], ID.Relu, bias=bias(L))
                ps2 = mm_ps.tile([128, G * 128], F32, name="ps2", tag="ps2")
                for j in range(nj):
                    nc.tensor.matmul(
                        ps2[:, j * 128 : (j + 1) * 128],
                        lhsT=r[:, j * 128 : (j + 1) * 128],
                        rhs=w_bf[:, (L + 1) * 128 : (L + 2) * 128],
                        start=True,
                        stop=True,
                        skip_group_check=True,
                    )
                for j, b in enumerate(blocks):
                    scale_table_block(b, ps2[:, j * 128 : (j + 1) * 128])
                flush_table_blocks(blocks[0], blocks[-1] + 1)
                for q in range(NQ - 1):
                    # early triggers for q0..q2; q3 is emitted lazily at the
                    # next layer's first gather so it doesn't block the
                    # gpsimd queue while the pipeline drains
                    if g == min(ag_ready_grp[q] + AG_LAG, ngrp - 1):
                        trigger_ag(L + 1, q)

        for L in range(3):
            pending = None
            for g in range(ngrp):
                st = agg_phase(L, g)
                if pending is not None:
                    epi_phase(pending)
                pending = st
            epi_phase(pending)

    nc.compile()
    return nc


_BUILD_CACHE = {}


def _get_kernel(sched):
    key = (
        sched["nblk"], sched["s_pad"],
        tuple(tuple(c) for c in sched["caps"]),
    )
    if key not in _BUILD_CACHE:
        _BUILD_CACHE[key] = build_kernel(sched)
    return _BUILD_CACHE[key]


# ----------------------------------------------------------------------------
# Entry point
# ----------------------------------------------------------------------------

def _run(x, edge_index, W0, b0, W1, b1, W2, b2, trace=False):
    n = int(np.asarray(x).shape[0])
    s_real = n // N_CORES
    in_maps, sched = _prep_inputs(
        x, edge_index, W0, b0, W1, b1, W2, b2, s_real
    )
    s_pad = sched["s_pad"]
    nc = _get_kernel(sched)
    res = bass_utils.run_bass_kernel_spmd(
        nc, in_maps, core_ids=list(range(N_CORES)), trace=trace
    )
    outs = []
    for L in range(3):
        h = np.concatenate(
            [
                np.asarray(
                    res.results[r]["h_out"][:, L * s_pad : L * s_pad + s_real],
                    dtype=np.float32,
                )
                for r in range(N_CORES)
            ],
            axis=1,
        ).T
        outs.append(h)
    full = np.stack(outs, axis=1).astype(np.float32)
    return full, res


def kernel(**inputs):
    trace = os.environ.get("TRN_KERNEL_TRACE", "") == "1"
    out, res = _run(
        np.asarray(inputs["x"]),
        np.asarray(inputs["edge_index"]),
        np.asarray(inputs["W0"]),
        np.asarray(inputs["b0"]),
        np.asarray(inputs["W1"]),
        np.asarray(inputs["b1"]),
        np.asarray(inputs["W2"]),
        np.asarray(inputs["b2"]),
        trace=trace,
    )
    if trace and res.exec_time_ns is not None:
        print(f"HW exec time: {res.exec_time_ns} ns")
        if res.instructions_and_trace:
            print(f"trace: {res.instructions_and_trace[1]}")
    return out


# revision 17
# speedup vs baseline: 1.7511x; 1.0325x over previous
"""3-layer GCN (gcn_norm message passing) on 8 Trainium2 NeuronCores.

Architecture (v5):
  - Nodes row-sharded across 8 cores (12500 real + 44 pad rows each). Per
    layer each core computes h_mm = relu(h_prev) @ W for its shard, scaled by
    dis[src] (norm = dis[dest]*dis[src] factorization), AllGathers the bf16
    table, then aggregates messages for the destinations it owns.
  - The table is laid out in 4 source-quarters, each quarter block-aligned
    within the core shard, so each layer runs FOUR AllGathers (one per
    quarter). A quarter's AG is triggered as soon as the groups producing its
    blocks finish, which overlaps collectives with the previous layer's tail
    and lets gathers start before the full table is assembled.
  - Messages (self-loops excluded) are sorted by (dest-group, source-quarter,
    dest-block, src) with exact per-(block, quarter) run capacities shared
    across cores (max over cores), one dma_gather per (group, quarter) on 4
    SWDGE queues (int16 indices into bf16 quarter windows <= 25600 rows).
  - Segment-sum on the TensorEngine: per dest block an identity matmul first
    adds the self-loop term from the SBUF-resident table, then each 128-slot
    chunk contributes one N=128 matmul per dest block it overlaps (q-major
    order) into a [128, 512] group PSUM tile. One-hots are built 16 chunks at
    a time with wide bf16 DVE tensor_tensor (iota == seg broadcast).
  - Group epilogue: ob = psum * dis[dest] (DVE, bf16 scale), +bias -> bf16
    h_out (ACT), relu (ACT), next-layer matmuls (PE), table write with
    dis[own] scale (ACT) into SBUF tbl + DMA to the quarter AG input.

All data-dependent structure is baked at trace time; the NEFF is compiled
per call and cached in-process.
"""

import os
import sys

sys.path.insert(0, "/opt/trn_rl_repo")

import numpy as np

from concourse import bacc, bass, mybir
from concourse import tile
from concourse import bass_utils

F32 = mybir.dt.float32
BF16 = mybir.dt.bfloat16
I16 = mybir.dt.int16

N_CORES = 8
NQ = 4       # source quarters (int16 windows / AllGather splits)
G = 4        # dest blocks per gather group
WOH = 16     # one-hot chunks per wide DVE tensor_tensor
PAD_SEG = 10000.0
AG_LAG = 2   # groups of slack before emitting a quarter's AG trigger


def _quarters(nblk):
    """Block-aligned source quarters: q -> (first block, n blocks)."""
    base, rem = divmod(nblk, NQ)
    sizes = [base + (1 if i < rem else 0) for i in range(NQ)]
    starts = np.concatenate([[0], np.cumsum(sizes)[:-1]]).astype(np.int64)
    return starts, np.asarray(sizes, dtype=np.int64)


def _schedule(caps, ngrp, nblk):
    """Static layout shared by prep and builder.

    caps: [nblk][NQ] per-(block, quarter) run capacity in slots (exact,
    max over cores). Returns per-call offsets and the chunk->block matmul
    schedule in (quarter, block, chunk) order.
    """
    call_cols = np.zeros(ngrp * NQ + 1, dtype=np.int64)   # gidx col base
    chunk_base = np.zeros(ngrp * NQ + 1, dtype=np.int64)  # chunk id base
    call_nidx = []
    run_slot = {}   # (b, q) -> slot offset of run inside its call
    mm_of_group = []
    for g in range(ngrp):
        blocks = list(range(g * G, min((g + 1) * G, nblk)))
        mms = []
        for q in range(NQ):
            off = 0
            spans = []
            for b in blocks:
                run_slot[(b, q)] = off
                spans.append((b, off, off + caps[b][q]))
                off += caps[b][q]
            nidx = ((off + 127) // 128) * 128
            call_nidx.append(nidx)
            call_cols[g * NQ + q + 1] = call_cols[g * NQ + q] + nidx // 16
            chunk_base[g * NQ + q + 1] = chunk_base[g * NQ + q] + nidx // 128
            for c in range(nidx // 128):
                lo, hi = c * 128, (c + 1) * 128
                for b, s0, s1 in spans:
                    if s0 < hi and s1 > lo:
                        mms.append((b - g * G, q, c))
        # q-major so quarter q's matmuls can start before later quarters land
        mms.sort(key=lambda m: (m[1], m[0], m[2]))
        mm_of_group.append(mms)
    mm_base = np.zeros(ngrp + 1, dtype=np.int64)
    np.cumsum([len(m) for m in mm_of_group], out=mm_base[1:])
    return {
        "call_cols": call_cols, "chunk_base": chunk_base,
        "call_nidx": call_nidx, "run_slot": run_slot,
        "mm_of_group": mm_of_group, "mm_base": mm_base,
        "n_mm": int(mm_base[-1]),
        "gidx_cols": int(call_cols[-1]),
        "total_chunks": int(chunk_base[-1]),
    }


# ----------------------------------------------------------------------------
# Host-side preparation
# ----------------------------------------------------------------------------

def _prep_inputs(x, edge_index, W0, b0, W1, b1, W2, b2, s_real):
    import ml_dtypes

    n = x.shape[0]
    assert n % N_CORES == 0 and s_real == n // N_CORES
    nblk = (s_real + 127) // 128
    s_pad = nblk * 128
    ngrp = (nblk + G - 1) // G
    qstart_b, qsize_b = _quarters(nblk)           # in blocks
    qstart = qstart_b * 128                        # local row starts
    qsize = qsize_b * 128                          # local rows per quarter
    assert all(int(s) * N_CORES <= 32767 for s in qsize)

    d = np.asarray(edge_index[0], dtype=np.int64)
    s = np.asarray(edge_index[1], dtype=np.int64)

    deg = np.bincount(s, minlength=n).astype(np.float64) + 1.0
    dis = (1.0 / np.sqrt(deg)).astype(np.float32)

    # self-loops handled by identity matmuls, not gathered messages
    core = d // s_real
    dloc = d - core * s_real
    blk = dloc >> 7
    grp = blk // G
    jj = blk - grp * G
    score = s // s_real
    sloc = s - score * s_real
    q = np.searchsorted(qstart, sloc, side="right") - 1
    widx = (score * qsize[q] + (sloc - qstart[q])).astype(np.int64)

    # per-(core, block, quarter) counts -> shared exact run capacities
    key = (core * nblk + blk) * NQ + q
    counts = np.bincount(key, minlength=N_CORES * nblk * NQ).reshape(
        N_CORES, nblk, NQ
    )
    caps = np.maximum(counts.max(axis=0), 1)  # [nblk, NQ]

    lay = _schedule(caps.tolist(), ngrp, nblk)

    # rank within (core, b, q), sorted by src for HBM locality
    order = np.lexsort((widx, key))
    inv = np.empty_like(order)
    inv[order] = np.arange(order.size)
    starts = np.zeros(N_CORES * nblk * NQ + 1, dtype=np.int64)
    np.cumsum(counts.reshape(-1), out=starts[1:])
    rank = inv - starts[key]

    run_slot_arr = np.zeros((nblk, NQ), dtype=np.int64)
    for (b, qq), v in lay["run_slot"].items():
        run_slot_arr[b, qq] = v

    slot = run_slot_arr[blk, q] + rank               # slot within call
    call_id = grp * NQ + q
    gcol = lay["call_cols"][call_id] + (slot >> 4)
    grow = slot & 15
    gchunk = lay["chunk_base"][call_id] + (slot >> 7)
    part = slot & 127

    gidx16 = np.zeros((N_CORES, 16, lay["gidx_cols"]), dtype=np.int16)
    gidx16[core, grow, gcol] = widx.astype(np.int16)
    gidx = np.broadcast_to(
        gidx16[:, None, :, :], (N_CORES, 8, 16, lay["gidx_cols"])
    ).reshape(N_CORES, 128, lay["gidx_cols"]).copy()

    # mm col lookup: (gchunk, j) -> column
    mm_col = np.full((lay["total_chunks"], G), -1, dtype=np.int64)
    for g in range(ngrp):
        m0 = lay["mm_base"][g]
        cb = lay["chunk_base"]
        for k, (j, qq, c) in enumerate(lay["mm_of_group"][g]):
            mm_col[cb[g * NQ + qq] + c, j] = m0 + k

    meta = np.full((N_CORES, 128, lay["n_mm"]), PAD_SEG, dtype=np.float32)
    col = mm_col[gchunk, jj]
    assert (col >= 0).all()
    meta[core, part, col] = (dloc - blk * 128).astype(np.float32)
    meta = meta.astype(ml_dtypes.bfloat16)

    # dense inputs
    x = np.asarray(x, dtype=np.float32)
    x_t = np.zeros((N_CORES, 128, s_pad), dtype=np.float32)
    dison = np.zeros((N_CORES, 128, nblk), dtype=np.float32)
    disd = np.zeros((N_CORES, 128, s_pad), dtype=np.float32)
    for r in range(N_CORES):
        x_t[r, :, :s_real] = x[r * s_real : (r + 1) * s_real].T
        dv = np.zeros(s_pad, dtype=np.float32)
        dv[:s_real] = dis[r * s_real : (r + 1) * s_real]
        dison[r] = dv.reshape(nblk, 128).T
        disd[r] = dv[None, :]
    x_t = x_t.astype(ml_dtypes.bfloat16)
    disd = disd.astype(ml_dtypes.bfloat16)

    wdata = np.zeros((128, 3 * 128 + 3), dtype=np.float32)
    wdata[:, 0:128] = np.asarray(W0, dtype=np.float32)
    wdata[:, 128:256] = np.asarray(W1, dtype=np.float32)
    wdata[:, 256:384] = np.asarray(W2, dtype=np.float32)
    wdata[:, 384] = np.asarray(b0, dtype=np.float32)
    wdata[:, 385] = np.asarray(b1, dtype=np.float32)
    wdata[:, 386] = np.asarray(b2, dtype=np.float32)
    iotar = np.tile(
        np.arange(128, dtype=np.float32), WOH
    )[None, :].repeat(128, axis=0).astype(ml_dtypes.bfloat16)
    ident = np.eye(128, dtype=np.float32).astype(ml_dtypes.bfloat16)

    in_maps = [
        {
            "x_t": x_t[r], "meta": meta[r], "gidx": gidx[r],
            "wdata": wdata, "iotar": iotar, "dison": dison[r],
            "disd": disd[r], "ident": ident,
        }
        for r in range(N_CORES)
    ]
    sched = {
        "nblk": nblk, "s_pad": s_pad, "s_real": s_real, "ngrp": ngrp,
        "caps": caps.tolist(),
    }
    return in_maps, sched


# ----------------------------------------------------------------------------
# Device kernel builder
# ----------------------------------------------------------------------------

def build_kernel(sched, n_cores=N_CORES):
    from contextlib import ExitStack

    nblk, s_pad, ngrp = sched["nblk"], sched["s_pad"], sched["ngrp"]
    caps = sched["caps"]
    lay = _schedule(caps, ngrp, nblk)
    qstart_b, qsize_b = _quarters(nblk)
    qstart_b = [int(v) for v in qstart_b]
    qsize_b = [int(v) for v in qsize_b]
    qend_b = [qstart_b[i] + qsize_b[i] - 1 for i in range(NQ)]  # last block
    # group after which quarter q's table blocks are all written
    ag_ready_grp = [qend_b[i] // G for i in range(NQ)]

    nc = bacc.Bacc(
        "TRN2", target_bir_lowering=False, debug=False, num_devices=n_cores,
        num_swdge_queues=NQ,
        # default 16 KiB ring overflows on ~2500-descriptor gather calls:
        # generation then blocks on ring space and head-of-line-stalls the
        # other queues' feeding. 32 KiB holds ~2 calls per queue.
        dynamic_dma_scratch_size=32768,
    )
    x_t = nc.dram_tensor("x_t", [128, s_pad], BF16, kind="ExternalInput")
    meta = nc.dram_tensor("meta", [128, lay["n_mm"]], BF16, kind="ExternalInput")
    gidx = nc.dram_tensor("gidx", [128, lay["gidx_cols"]], I16, kind="ExternalInput")
    wdata = nc.dram_tensor("wdata", [128, 3 * 128 + 3], F32, kind="ExternalInput")
    iotar = nc.dram_tensor("iotar", [128, WOH * 128], BF16, kind="ExternalInput")
    dison = nc.dram_tensor("dison", [128, nblk], F32, kind="ExternalInput")
    disd = nc.dram_tensor("disd", [128, s_pad], BF16, kind="ExternalInput")
    ident_in = nc.dram_tensor("ident", [128, 128], BF16, kind="ExternalInput")
    h_out = nc.dram_tensor("h_out", [128, 3 * s_pad], BF16, kind="ExternalOutput")

    rg = [list(range(n_cores))]
    ID = mybir.ActivationFunctionType

    with tile.TileContext(nc) as tc, ExitStack() as ctx:
        const = ctx.enter_context(tc.tile_pool(name="const", bufs=1))
        dram = ctx.enter_context(tc.tile_pool(name="dram", bufs=1, space="DRAM"))
        tblp = ctx.enter_context(tc.tile_pool(name="tblp", bufs=1))
        xw = ctx.enter_context(tc.tile_pool(name="xw", bufs=2))
        gath = ctx.enter_context(tc.tile_pool(name="gath", bufs=10))
        idxp = ctx.enter_context(tc.tile_pool(name="idxp", bufs=10))
        metat = ctx.enter_context(tc.tile_pool(name="metat", bufs=4))
        ohp = ctx.enter_context(tc.tile_pool(name="ohp", bufs=8))
        outsb = ctx.enter_context(tc.tile_pool(name="outsb", bufs=3))
        obbf = ctx.enter_context(tc.tile_pool(name="obbf", bufs=3))
        rsb = ctx.enter_context(tc.tile_pool(name="rsb", bufs=3))
        agg_ps = ctx.enter_context(tc.tile_pool(name="agg_ps", bufs=3, space="PSUM"))
        mm_ps = ctx.enter_context(tc.tile_pool(name="mm_ps", bufs=2, space="PSUM"))
        mma_ps = ctx.enter_context(tc.tile_pool(name="mma_ps", bufs=2, space="PSUM"))

        ag_ins = [
            dram.tile([qsize_b[q] * 128, 128], BF16, name=f"ag_in_q{q}")
            for q in range(NQ)
        ]
        ag_outs = [
            [
                dram.tile(
                    [n_cores * qsize_b[q] * 128, 128], BF16,
                    addr_space="Shared", name=f"ag_out_l{i}_q{q}",
                )
                for q in range(NQ)
            ]
            for i in range(3)
        ]

        w_sb = const.tile([128, 3 * 128 + 3], F32)
        nc.sync.dma_start(out=w_sb[:], in_=wdata[:])
        w_bf = const.tile([128, 3 * 128], BF16)
        nc.vector.tensor_copy(w_bf[:], w_sb[:, 0 : 3 * 128])
        iota_sb = const.tile([128, WOH * 128], BF16)
        nc.sync.dma_start(out=iota_sb[:], in_=iotar[:])
        dison_sb = const.tile([128, nblk], F32)
        nc.sync.dma_start(out=dison_sb[:], in_=dison[:])
        dd_sb = const.tile([128, s_pad], BF16)
        nc.sync.dma_start(out=dd_sb[:], in_=disd[:])
        ident = const.tile([128, 128], BF16)
        nc.sync.dma_start(out=ident[:], in_=ident_in[:])

        tbl = tblp.tile([128, s_pad], BF16)

        def bias(L):
            return w_sb[:, 384 + L : 385 + L]

        def blk_quarter(b):
            for q in range(NQ):
                if qstart_b[q] <= b <= qend_b[q]:
                    return q
            raise AssertionError(b)

        def scale_table_block(b, src_ps):
            """ACT: tbl[:, b] = src_ps * dis[own block b]."""
            nc.scalar.activation(
                tbl[:, b * 128 : (b + 1) * 128], src_ps, ID.Copy,
                scale=dison_sb[:, b : b + 1],
            )

        def flush_table_blocks(b0, b1):
            """DMA tbl blocks [b0, b1) to AG inputs, split on quarter edges.

            The DRAM rows interleave blocks (node = block*128 + partition), so
            both sides use matching [partition, block, feat] access patterns.
            """
            b = b0
            while b < b1:
                q = blk_quarter(b)
                e = min(b1, qend_b[q] + 1)
                r0 = (b - qstart_b[q]) * 128
                n = e - b
                nc.scalar.dma_start(
                    out=ag_ins[q][r0 : r0 + n * 128, :].rearrange(
                        "(k i) f -> i k f", i=128
                    ),
                    in_=tbl[:, b * 128 : e * 128].rearrange(
                        "p (k f) -> p k f", f=128
                    ),
                )
                b = e

        ag_done = set()

        def trigger_ag(L, q):
            if (L, q) in ag_done:
                return
            ag_done.add((L, q))
            nc.gpsimd.collective_compute(
                "AllGather",
                mybir.AluOpType.bypass,
                replica_groups=rg,
                ins=[ag_ins[q][:].opt()],
                outs=[ag_outs[L][q][:].opt()],
            )

        # warmup collective: absorbs the first-collective barrier/setup cost
        # concurrently with Phase A instead of delaying the first real AG
        warm_in = dram.tile([128, 128], BF16, name="warm_in")
        warm_out = dram.tile([n_cores * 128, 128], BF16,
                             addr_space="Shared", name="warm_out")
        nc.sync.dma_start(out=warm_in[:], in_=ident_in[:])
        nc.gpsimd.collective_compute(
            "AllGather",
            mybir.AluOpType.bypass,
            replica_groups=rg,
            ins=[warm_in[:].opt()],
            outs=[warm_out[:].opt()],
        )

        # ---- Phase A: table0 = (x @ W0) * dis -> tbl + ag_in ----
        XW_BLKS = 16
        FLUSH = 8
        flushed = 0
        for b in range(nblk):
            if b % XW_BLKS == 0:
                xt = xw.tile([128, XW_BLKS * 128], BF16, name="xt", tag="xt")
                w = min(XW_BLKS * 128, s_pad - b * 128)
                nc.sync.dma_start(
                    out=xt[:, :w], in_=x_t[:, b * 128 : b * 128 + w]
                )
            o = (b % XW_BLKS) * 128
            ps = mma_ps.tile([128, 128], F32, name="psA", tag="psA")
            nc.tensor.matmul(
                ps[:], lhsT=xt[:, o : o + 128], rhs=w_bf[:, 0:128],
                start=True, stop=True,
            )
            scale_table_block(b, ps[:])
            if b + 1 - flushed >= FLUSH or b == nblk - 1 or b in qend_b:
                flush_table_blocks(flushed, b + 1)
                flushed = b + 1
            for q in range(NQ - 1):
                # lag a couple of blocks so the AG trigger's deps are done;
                # q3's trigger is emitted lazily after the first few gathers
                if b == min(qend_b[q] + 2, nblk - 1):
                    trigger_ag(0, q)

        # ---- 3 layers, software-pipelined emission: the aggregation of
        # group g+1 is emitted before the epilogue of group g so neither the
        # DVE (one-hots vs psum-read) nor the PE (chunk mms vs ps2 mms)
        # head-of-line blocks on the other group's dependencies.
        def gather_call(L, g, q):
            trigger_ag(L, q)  # no-op unless not yet emitted (lazy)
            nidx = lay["call_nidx"][g * NQ + q]
            c0 = lay["call_cols"][g * NQ + q]
            it = idxp.tile([128, nidx // 16], I16, name="it", tag="it")
            nc.sync.dma_start(
                out=it[:], in_=gidx[:, c0 : c0 + nidx // 16]
            )
            gt = gath.tile([128, nidx], BF16, name="gt", tag="gt")
            nc.gpsimd.dma_gather(
                gt[:].rearrange("p (c f) -> p c f", f=128),
                ag_outs[L][q][:],
                it[:],
                num_idxs=nidx,
                num_idxs_reg=nidx,
                elem_size=128,
                elem_step=128,
                single_packet=(nidx <= 1024),
                queue_num=q,
            )
            return gt

        def agg_phase(L, g, pre):
            blocks = list(range(g * G, min((g + 1) * G, nblk)))
            gts = [
                pre.pop((g, q), None) or gather_call(L, g, q)
                for q in range(NQ)
            ]

            mms = lay["mm_of_group"][g]
            m0 = int(lay["mm_base"][g])
            n_mm_g = len(mms)
            mt = metat.tile([128, n_mm_g], BF16)
            nc.sync.dma_start(out=mt[:], in_=meta[:, m0 : m0 + n_mm_g])

            ohs = {}
            for w0 in range(0, n_mm_g, WOH):
                wn = min(WOH, n_mm_g - w0)
                oh = ohp.tile([128, wn * 128], BF16, name="oh", tag="oh")
                nc.vector.tensor_tensor(
                    oh[:].rearrange("p (c f) -> p c f", f=128),
                    iota_sb[:, : wn * 128].rearrange("p (c f) -> p c f", f=128),
                    mt[:, w0 : w0 + wn].to_broadcast([128, wn, 128]),
                    mybir.AluOpType.is_equal,
                )
                ohs[w0] = oh

            ps = agg_ps.tile([128, G * 128], F32, name="aggps", tag="aggps")
            # ONE start per PSUM bank (start=True clears has_written for
            # the whole bank): only the very first matmul starts, only the
            # very last stops; each element's first writer overwrites.
            for j, b in enumerate(blocks):
                nc.tensor.matmul(
                    ps[:, j * 128 : (j + 1) * 128],
                    lhsT=tbl[:, b * 128 : (b + 1) * 128],
                    rhs=ident[:],
                    start=(j == 0),
                    stop=False,
                    skip_group_check=True,
                )
            for k, (j, qq, c) in enumerate(mms):
                w0 = (k // WOH) * WOH
                off = k - w0
                nc.tensor.matmul(
                    ps[:, j * 128 : (j + 1) * 128],
                    lhsT=gts[qq][:, c * 128 : (c + 1) * 128],
                    rhs=ohs[w0][:, off * 128 : (off + 1) * 128],
                    start=False,
                    stop=(k == n_mm_g - 1),
                    skip_group_check=True,
                )
            return (L, g, blocks, ps)

        def epi_phase(st):
            L, g, blocks, ps = st
            nj = len(blocks)
            w = nj * 128
            gb = g * G * 128
            ob = outsb.tile([128, G * 128], F32, name="ob", tag="ob")
            nc.vector.tensor_tensor(
                ob[:, :w], ps[:, :w], dd_sb[:, gb : gb + w],
                mybir.AluOpType.mult,
            )
            obf = obbf.tile([128, G * 128], BF16, name="obf", tag="obf")
            nc.scalar.activation(
                obf[:, :w], ob[:, :w], ID.Identity, bias=bias(L)
            )
            nc.sync.dma_start(
                out=h_out[:, L * s_pad + gb : L * s_pad + gb + w],
                in_=obf[:, :w],
            )
            if L < 2:
                r = rsb.tile([128, G * 128], BF16, name="r", tag="r")
                nc.scalar.activation(r[:, :w], ob[:, :w], ID.Relu, bias=bias(L))
                ps2 = mm_ps.tile([128, G * 128], F32, name="ps2", tag="ps2")
                for j in range(nj):
                    nc.tensor.matmul(
                        ps2[:, j * 128 : (j + 1) * 128],
                        lhsT=r[:, j * 128 : (j + 1) * 128],
                        rhs=w_bf[:, (L + 1) * 128 : (L + 2) * 128],
                        start=True,
                        stop=True,
                        skip_group_check=True,
                    )
                for j, b in enumerate(blocks):
                    scale_table_block(b, ps2[:, j * 128 : (j + 1) * 128])
                flush_table_blocks(blocks[0], blocks[-1] + 1)
                for q in range(NQ - 1):
                    # early triggers for q0..q2; q3 is emitted lazily at the
                    # next layer's first gather so it doesn't block the
                    # gpsimd queue while the pipeline drains
                    if g == min(ag_ready_grp[q] + AG_LAG, ngrp - 1):
                        trigger_ag(L + 1, q)

        DEFER = 2  # groups whose q0-q2 gathers are emitted before the first
        # q3 gather, so AG_q3 (triggered last, at the previous layer's tail)
        # overlaps with useful generation + drain instead of stalling gpsimd
        for L in range(3):
            pending = None
            pre = {}
            for g in range(min(DEFER, ngrp)):
                for q in range(NQ - 1):
                    pre[(g, q)] = gather_call(L, g, q)
            for g in range(ngrp):
                st = agg_phase(L, g, pre)
                if pending is not None:
                    epi_phase(pending)
                pending = st
            epi_phase(pending)

    nc.compile()
    return nc


_BUILD_CACHE = {}


def _get_kernel(sched):
    key = (
        sched["nblk"], sched["s_pad"],
        tuple(tuple(c) for c in sched["caps"]),
    )
    if key not in _BUILD_CACHE:
        _BUILD_CACHE[key] = build_kernel(sched)
    return _BUILD_CACHE[key]


# ----------------------------------------------------------------------------
# Entry point
# ----------------------------------------------------------------------------

def _run(x, edge_index, W0, b0, W1, b1, W2, b2, trace=False):
    n = int(np.asarray(x).shape[0])
    s_real = n // N_CORES
    in_maps, sched = _prep_inputs(
        x, edge_index, W0, b0, W1, b1, W2, b2, s_real
    )
    s_pad = sched["s_pad"]
    nc = _get_kernel(sched)
    res = bass_utils.run_bass_kernel_spmd(
        nc, in_maps, core_ids=list(range(N_CORES)), trace=trace
    )
    outs = []
    for L in range(3):
        h = np.concatenate(
            [
                np.asarray(
                    res.results[r]["h_out"][:, L * s_pad : L * s_pad + s_real],
                    dtype=np.float32,
                )
                for r in range(N_CORES)
            ],
            axis=1,
        ).T
        outs.append(h)
    full = np.stack(outs, axis=1).astype(np.float32)
    return full, res


def kernel(**inputs):
    trace = os.environ.get("TRN_KERNEL_TRACE", "") == "1"
    out, res = _run(
        np.asarray(inputs["x"]),
        np.asarray(inputs["edge_index"]),
        np.asarray(inputs["W0"]),
        np.asarray(inputs["b0"]),
        np.asarray(inputs["W1"]),
        np.asarray(inputs["b1"]),
        np.asarray(inputs["W2"]),
        np.asarray(inputs["b2"]),
        trace=trace,
    )
    if trace and res.exec_time_ns is not None:
        print(f"HW exec time: {res.exec_time_ns} ns")
        if res.instructions_and_trace:
            print(f"trace: {res.instructions_and_trace[1]}")
    return out


# revision 18
# speedup vs baseline: 1.8327x; 1.0466x over previous
"""3-layer GCN (gcn_norm message passing) on 8 Trainium2 NeuronCores.

Architecture (v5):
  - Nodes row-sharded across 8 cores (12500 real + 44 pad rows each). Per
    layer each core computes h_mm = relu(h_prev) @ W for its shard, scaled by
    dis[src] (norm = dis[dest]*dis[src] factorization), AllGathers the bf16
    table, then aggregates messages for the destinations it owns.
  - The table is laid out in 4 source-quarters, each quarter block-aligned
    within the core shard, so each layer runs FOUR AllGathers (one per
    quarter). A quarter's AG is triggered as soon as the groups producing its
    blocks finish, which overlaps collectives with the previous layer's tail
    and lets gathers start before the full table is assembled.
  - Messages (self-loops excluded) are sorted by (dest-group, source-quarter,
    dest-block, src) with exact per-(block, quarter) run capacities shared
    across cores (max over cores), one dma_gather per (group, quarter) on 4
    SWDGE queues (int16 indices into bf16 quarter windows <= 25600 rows).
  - Segment-sum on the TensorEngine: per dest block an identity matmul first
    adds the self-loop term from the SBUF-resident table, then each 128-slot
    chunk contributes one N=128 matmul per dest block it overlaps (q-major
    order) into a [128, 512] group PSUM tile. One-hots are built 16 chunks at
    a time with wide bf16 DVE tensor_tensor (iota == seg broadcast).
  - Group epilogue: ob = psum * dis[dest] (DVE, bf16 scale), +bias -> bf16
    h_out (ACT), relu (ACT), next-layer matmuls (PE), table write with
    dis[own] scale (ACT) into SBUF tbl + DMA to the quarter AG input.

All data-dependent structure is baked at trace time; the NEFF is compiled
per call and cached in-process.
"""

import os
import sys

sys.path.insert(0, "/opt/trn_rl_repo")

import numpy as np

from concourse import bacc, bass, mybir
from concourse import tile
from concourse import bass_utils

F32 = mybir.dt.float32
BF16 = mybir.dt.bfloat16
I16 = mybir.dt.int16

N_CORES = 8
NQ = 4       # source quarters (int16 windows / AllGather splits)
G = 4        # dest blocks per gather group
WOH = 16     # one-hot chunks per wide DVE tensor_tensor
PAD_SEG = 10000.0
AG_LAG = 2   # groups of slack before emitting a quarter's AG trigger


def _quarters(nblk):
    """Block-aligned source quarters: q -> (first block, n blocks)."""
    base, rem = divmod(nblk, NQ)
    sizes = [base + (1 if i < rem else 0) for i in range(NQ)]
    starts = np.concatenate([[0], np.cumsum(sizes)[:-1]]).astype(np.int64)
    return starts, np.asarray(sizes, dtype=np.int64)


def _schedule(caps, ngrp, nblk):
    """Static layout shared by prep and builder.

    caps: [nblk][NQ] per-(block, quarter) run capacity in slots (exact,
    max over cores). Returns per-call offsets and the chunk->block matmul
    schedule in (quarter, block, chunk) order.
    """
    call_cols = np.zeros(ngrp * NQ + 1, dtype=np.int64)   # gidx col base
    chunk_base = np.zeros(ngrp * NQ + 1, dtype=np.int64)  # chunk id base
    call_nidx = []
    run_slot = {}   # (b, q) -> slot offset of run inside its call
    mm_of_group = []
    for g in range(ngrp):
        blocks = list(range(g * G, min((g + 1) * G, nblk)))
        mms = []
        for q in range(NQ):
            off = 0
            spans = []
            for b in blocks:
                run_slot[(b, q)] = off
                spans.append((b, off, off + caps[b][q]))
                off += caps[b][q]
            nidx = ((off + 127) // 128) * 128
            call_nidx.append(nidx)
            call_cols[g * NQ + q + 1] = call_cols[g * NQ + q] + nidx // 16
            chunk_base[g * NQ + q + 1] = chunk_base[g * NQ + q] + nidx // 128
            for c in range(nidx // 128):
                lo, hi = c * 128, (c + 1) * 128
                for b, s0, s1 in spans:
                    if s0 < hi and s1 > lo:
                        mms.append((b - g * G, q, c))
        # q-major so quarter q's matmuls can start before later quarters land
        mms.sort(key=lambda m: (m[1], m[0], m[2]))
        mm_of_group.append(mms)
    mm_base = np.zeros(ngrp + 1, dtype=np.int64)
    np.cumsum([len(m) for m in mm_of_group], out=mm_base[1:])
    return {
        "call_cols": call_cols, "chunk_base": chunk_base,
        "call_nidx": call_nidx, "run_slot": run_slot,
        "mm_of_group": mm_of_group, "mm_base": mm_base,
        "n_mm": int(mm_base[-1]),
        "gidx_cols": int(call_cols[-1]),
        "total_chunks": int(chunk_base[-1]),
    }


# ----------------------------------------------------------------------------
# Host-side preparation
# ----------------------------------------------------------------------------

def _prep_inputs(x, edge_index, W0, b0, W1, b1, W2, b2, s_real):
    import ml_dtypes

    n = x.shape[0]
    assert n % N_CORES == 0 and s_real == n // N_CORES
    nblk = (s_real + 127) // 128
    s_pad = nblk * 128
    ngrp = (nblk + G - 1) // G
    qstart_b, qsize_b = _quarters(nblk)           # in blocks
    qstart = qstart_b * 128                        # local row starts
    qsize = qsize_b * 128                          # local rows per quarter
    assert all(int(s) * N_CORES <= 32767 for s in qsize)

    d = np.asarray(edge_index[0], dtype=np.int64)
    s = np.asarray(edge_index[1], dtype=np.int64)

    deg = np.bincount(s, minlength=n).astype(np.float64) + 1.0
    dis = (1.0 / np.sqrt(deg)).astype(np.float32)

    # self-loops handled by identity matmuls, not gathered messages
    core = d // s_real
    dloc = d - core * s_real
    blk = dloc >> 7
    grp = blk // G
    jj = blk - grp * G
    score = s // s_real
    sloc = s - score * s_real
    q = np.searchsorted(qstart, sloc, side="right") - 1
    widx = (score * qsize[q] + (sloc - qstart[q])).astype(np.int64)

    # per-(core, block, quarter) counts -> shared exact run capacities
    key = (core * nblk + blk) * NQ + q
    counts = np.bincount(key, minlength=N_CORES * nblk * NQ).reshape(
        N_CORES, nblk, NQ
    )
    caps = np.maximum(counts.max(axis=0), 1)  # [nblk, NQ]

    lay = _schedule(caps.tolist(), ngrp, nblk)

    # rank within (core, b, q), sorted by src for HBM locality
    order = np.lexsort((widx, key))
    inv = np.empty_like(order)
    inv[order] = np.arange(order.size)
    starts = np.zeros(N_CORES * nblk * NQ + 1, dtype=np.int64)
    np.cumsum(counts.reshape(-1), out=starts[1:])
    rank = inv - starts[key]

    run_slot_arr = np.zeros((nblk, NQ), dtype=np.int64)
    for (b, qq), v in lay["run_slot"].items():
        run_slot_arr[b, qq] = v

    slot = run_slot_arr[blk, q] + rank               # slot within call
    call_id = grp * NQ + q
    gcol = lay["call_cols"][call_id] + (slot >> 4)
    grow = slot & 15
    gchunk = lay["chunk_base"][call_id] + (slot >> 7)
    part = slot & 127

    gidx16 = np.zeros((N_CORES, 16, lay["gidx_cols"]), dtype=np.int16)
    gidx16[core, grow, gcol] = widx.astype(np.int16)
    gidx = np.broadcast_to(
        gidx16[:, None, :, :], (N_CORES, 8, 16, lay["gidx_cols"])
    ).reshape(N_CORES, 128, lay["gidx_cols"]).copy()

    # mm col lookup: (gchunk, j) -> column
    mm_col = np.full((lay["total_chunks"], G), -1, dtype=np.int64)
    for g in range(ngrp):
        m0 = lay["mm_base"][g]
        cb = lay["chunk_base"]
        for k, (j, qq, c) in enumerate(lay["mm_of_group"][g]):
            mm_col[cb[g * NQ + qq] + c, j] = m0 + k

    meta = np.full((N_CORES, 128, lay["n_mm"]), PAD_SEG, dtype=np.float32)
    col = mm_col[gchunk, jj]
    assert (col >= 0).all()
    meta[core, part, col] = (dloc - blk * 128).astype(np.float32)
    meta = meta.astype(ml_dtypes.bfloat16)

    # dense inputs
    x = np.asarray(x, dtype=np.float32)
    x_t = np.zeros((N_CORES, 128, s_pad), dtype=np.float32)
    dison = np.zeros((N_CORES, 128, nblk), dtype=np.float32)
    disd = np.zeros((N_CORES, 128, s_pad), dtype=np.float32)
    for r in range(N_CORES):
        x_t[r, :, :s_real] = x[r * s_real : (r + 1) * s_real].T
        dv = np.zeros(s_pad, dtype=np.float32)
        dv[:s_real] = dis[r * s_real : (r + 1) * s_real]
        dison[r] = dv.reshape(nblk, 128).T
        disd[r] = dv[None, :]
    x_t = x_t.astype(ml_dtypes.bfloat16)
    disd = disd.astype(ml_dtypes.bfloat16)

    wdata = np.zeros((128, 3 * 128 + 3), dtype=np.float32)
    wdata[:, 0:128] = np.asarray(W0, dtype=np.float32)
    wdata[:, 128:256] = np.asarray(W1, dtype=np.float32)
    wdata[:, 256:384] = np.asarray(W2, dtype=np.float32)
    wdata[:, 384] = np.asarray(b0, dtype=np.float32)
    wdata[:, 385] = np.asarray(b1, dtype=np.float32)
    wdata[:, 386] = np.asarray(b2, dtype=np.float32)
    iotar = np.tile(
        np.arange(128, dtype=np.float32), WOH
    )[None, :].repeat(128, axis=0).astype(ml_dtypes.bfloat16)
    ident = np.eye(128, dtype=np.float32).astype(ml_dtypes.bfloat16)

    in_maps = [
        {
            "x_t": x_t[r], "meta": meta[r], "gidx": gidx[r],
            "wdata": wdata, "iotar": iotar, "dison": dison[r],
            "disd": disd[r], "ident": ident,
        }
        for r in range(N_CORES)
    ]
    sched = {
        "nblk": nblk, "s_pad": s_pad, "s_real": s_real, "ngrp": ngrp,
        "caps": caps.tolist(),
    }
    return in_maps, sched


# ----------------------------------------------------------------------------
# Device kernel builder
# ----------------------------------------------------------------------------

def build_kernel(sched, n_cores=N_CORES):
    from contextlib import ExitStack

    nblk, s_pad, ngrp = sched["nblk"], sched["s_pad"], sched["ngrp"]
    caps = sched["caps"]
    lay = _schedule(caps, ngrp, nblk)
    qstart_b, qsize_b = _quarters(nblk)
    qstart_b = [int(v) for v in qstart_b]
    qsize_b = [int(v) for v in qsize_b]
    qend_b = [qstart_b[i] + qsize_b[i] - 1 for i in range(NQ)]  # last block
    # group after which quarter q's table blocks are all written
    ag_ready_grp = [qend_b[i] // G for i in range(NQ)]

    nc = bacc.Bacc(
        "TRN2", target_bir_lowering=False, debug=False, num_devices=n_cores,
        num_swdge_queues=NQ,
    )
    x_t = nc.dram_tensor("x_t", [128, s_pad], BF16, kind="ExternalInput")
    meta = nc.dram_tensor("meta", [128, lay["n_mm"]], BF16, kind="ExternalInput")
    gidx = nc.dram_tensor("gidx", [128, lay["gidx_cols"]], I16, kind="ExternalInput")
    wdata = nc.dram_tensor("wdata", [128, 3 * 128 + 3], F32, kind="ExternalInput")
    iotar = nc.dram_tensor("iotar", [128, WOH * 128], BF16, kind="ExternalInput")
    dison = nc.dram_tensor("dison", [128, nblk], F32, kind="ExternalInput")
    disd = nc.dram_tensor("disd", [128, s_pad], BF16, kind="ExternalInput")
    ident_in = nc.dram_tensor("ident", [128, 128], BF16, kind="ExternalInput")
    h_out = nc.dram_tensor("h_out", [128, 3 * s_pad], BF16, kind="ExternalOutput")

    rg = [list(range(n_cores))]
    ID = mybir.ActivationFunctionType

    with tile.TileContext(nc) as tc, ExitStack() as ctx:
        const = ctx.enter_context(tc.tile_pool(name="const", bufs=1))
        dram = ctx.enter_context(tc.tile_pool(name="dram", bufs=1, space="DRAM"))
        tblp = ctx.enter_context(tc.tile_pool(name="tblp", bufs=1))
        xw = ctx.enter_context(tc.tile_pool(name="xw", bufs=2))
        gath = ctx.enter_context(tc.tile_pool(name="gath", bufs=10))
        idxp = ctx.enter_context(tc.tile_pool(name="idxp", bufs=10))
        metat = ctx.enter_context(tc.tile_pool(name="metat", bufs=4))
        ohp = ctx.enter_context(tc.tile_pool(name="ohp", bufs=14))
        outsb = ctx.enter_context(tc.tile_pool(name="outsb", bufs=3))
        obbf = ctx.enter_context(tc.tile_pool(name="obbf", bufs=3))
        rsb = ctx.enter_context(tc.tile_pool(name="rsb", bufs=3))
        agg_ps = ctx.enter_context(tc.tile_pool(name="agg_ps", bufs=3, space="PSUM"))
        mm_ps = ctx.enter_context(tc.tile_pool(name="mm_ps", bufs=2, space="PSUM"))
        mma_ps = ctx.enter_context(tc.tile_pool(name="mma_ps", bufs=2, space="PSUM"))

        ag_ins = [
            dram.tile([qsize_b[q] * 128, 128], BF16, name=f"ag_in_q{q}")
            for q in range(NQ)
        ]
        ag_outs = [
            [
                dram.tile(
                    [n_cores * qsize_b[q] * 128, 128], BF16,
                    addr_space="Shared", name=f"ag_out_l{i}_q{q}",
                )
                for q in range(NQ)
            ]
            for i in range(3)
        ]

        w_sb = const.tile([128, 3 * 128 + 3], F32)
        nc.sync.dma_start(out=w_sb[:], in_=wdata[:])
        w_bf = const.tile([128, 3 * 128], BF16)
        nc.vector.tensor_copy(w_bf[:], w_sb[:, 0 : 3 * 128])
        iota_sb = const.tile([128, WOH * 128], BF16)
        nc.sync.dma_start(out=iota_sb[:], in_=iotar[:])
        dison_sb = const.tile([128, nblk], F32)
        nc.sync.dma_start(out=dison_sb[:], in_=dison[:])
        dd_sb = const.tile([128, s_pad], BF16)
        nc.sync.dma_start(out=dd_sb[:], in_=disd[:])
        ident = const.tile([128, 128], BF16)
        nc.sync.dma_start(out=ident[:], in_=ident_in[:])

        tbl = tblp.tile([128, s_pad], BF16)

        def bias(L):
            return w_sb[:, 384 + L : 385 + L]

        def blk_quarter(b):
            for q in range(NQ):
                if qstart_b[q] <= b <= qend_b[q]:
                    return q
            raise AssertionError(b)

        def scale_table_block(b, src_ps):
            """ACT: tbl[:, b] = src_ps * dis[own block b]."""
            nc.scalar.activation(
                tbl[:, b * 128 : (b + 1) * 128], src_ps, ID.Copy,
                scale=dison_sb[:, b : b + 1],
            )

        def flush_table_blocks(b0, b1):
            """DMA tbl blocks [b0, b1) to AG inputs, split on quarter edges.

            The DRAM rows interleave blocks (node = block*128 + partition), so
            both sides use matching [partition, block, feat] access patterns.
            """
            b = b0
            while b < b1:
                q = blk_quarter(b)
                e = min(b1, qend_b[q] + 1)
                r0 = (b - qstart_b[q]) * 128
                n = e - b
                nc.scalar.dma_start(
                    out=ag_ins[q][r0 : r0 + n * 128, :].rearrange(
                        "(k i) f -> i k f", i=128
                    ),
                    in_=tbl[:, b * 128 : e * 128].rearrange(
                        "p (k f) -> p k f", f=128
                    ),
                )
                b = e

        ag_done = set()

        def trigger_ag(L, q):
            if (L, q) in ag_done:
                return
            ag_done.add((L, q))
            nc.gpsimd.collective_compute(
                "AllGather",
                mybir.AluOpType.bypass,
                replica_groups=rg,
                ins=[ag_ins[q][:].opt()],
                outs=[ag_outs[L][q][:].opt()],
            )

        # warmup collective: absorbs the first-collective barrier/setup cost
        # concurrently with Phase A instead of delaying the first real AG
        warm_in = dram.tile([128, 128], BF16, name="warm_in")
        warm_out = dram.tile([n_cores * 128, 128], BF16,
                             addr_space="Shared", name="warm_out")
        nc.sync.dma_start(out=warm_in[:], in_=ident_in[:])
        nc.gpsimd.collective_compute(
            "AllGather",
            mybir.AluOpType.bypass,
            replica_groups=rg,
            ins=[warm_in[:].opt()],
            outs=[warm_out[:].opt()],
        )

        # ---- Phase A: table0 = (x @ W0) * dis -> tbl + ag_in ----
        XW_BLKS = 8
        FLUSH = 8
        flushed = 0
        for b in range(nblk):
            if b % XW_BLKS == 0:
                xt = xw.tile([128, XW_BLKS * 128], BF16, name="xt", tag="xt")
                w = min(XW_BLKS * 128, s_pad - b * 128)
                nc.sync.dma_start(
                    out=xt[:, :w], in_=x_t[:, b * 128 : b * 128 + w]
                )
            o = (b % XW_BLKS) * 128
            ps = mma_ps.tile([128, 128], F32, name="psA", tag="psA")
            nc.tensor.matmul(
                ps[:], lhsT=xt[:, o : o + 128], rhs=w_bf[:, 0:128],
                start=True, stop=True,
            )
            scale_table_block(b, ps[:])
            if b + 1 - flushed >= FLUSH or b == nblk - 1 or b in qend_b:
                flush_table_blocks(flushed, b + 1)
                flushed = b + 1
            for q in range(NQ - 1):
                # lag a couple of blocks so the AG trigger's deps are done;
                # q3's trigger is emitted lazily after the first few gathers
                if b == min(qend_b[q] + 2, nblk - 1):
                    trigger_ag(0, q)

        # ---- 3 layers, software-pipelined emission: the aggregation of
        # group g+1 is emitted before the epilogue of group g so neither the
        # DVE (one-hots vs psum-read) nor the PE (chunk mms vs ps2 mms)
        # head-of-line blocks on the other group's dependencies.
        def gather_call(L, g, q):
            trigger_ag(L, q)  # no-op unless not yet emitted (lazy)
            nidx = lay["call_nidx"][g * NQ + q]
            c0 = lay["call_cols"][g * NQ + q]
            it = idxp.tile([128, nidx // 16], I16, name="it", tag="it")
            nc.sync.dma_start(
                out=it[:], in_=gidx[:, c0 : c0 + nidx // 16]
            )
            gt = gath.tile([128, nidx], BF16, name="gt", tag="gt")
            nc.gpsimd.dma_gather(
                gt[:].rearrange("p (c f) -> p c f", f=128),
                ag_outs[L][q][:],
                it[:],
                num_idxs=nidx,
                num_idxs_reg=nidx,
                elem_size=128,
                elem_step=128,
                single_packet=(nidx <= 1024),
                queue_num=q,
            )
            return gt

        def agg_phase(L, g, pre):
            blocks = list(range(g * G, min((g + 1) * G, nblk)))
            gts = [
                pre.pop((g, q), None) or gather_call(L, g, q)
                for q in range(NQ)
            ]

            mms = lay["mm_of_group"][g]
            m0 = int(lay["mm_base"][g])
            n_mm_g = len(mms)
            mt = metat.tile([128, n_mm_g], BF16)
            nc.sync.dma_start(out=mt[:], in_=meta[:, m0 : m0 + n_mm_g])

            ohs = {}
            for w0 in range(0, n_mm_g, WOH):
                wn = min(WOH, n_mm_g - w0)
                oh = ohp.tile([128, wn * 128], BF16, name="oh", tag="oh")
                nc.vector.tensor_tensor(
                    oh[:].rearrange("p (c f) -> p c f", f=128),
                    iota_sb[:, : wn * 128].rearrange("p (c f) -> p c f", f=128),
                    mt[:, w0 : w0 + wn].to_broadcast([128, wn, 128]),
                    mybir.AluOpType.is_equal,
                )
                ohs[w0] = oh

            ps = agg_ps.tile([128, G * 128], F32, name="aggps", tag="aggps")
            # ONE start per PSUM bank (start=True clears has_written for
            # the whole bank): only the very first matmul starts, only the
            # very last stops; each element's first writer overwrites.
            for j, b in enumerate(blocks):
                nc.tensor.matmul(
                    ps[:, j * 128 : (j + 1) * 128],
                    lhsT=tbl[:, b * 128 : (b + 1) * 128],
                    rhs=ident[:],
                    start=(j == 0),
                    stop=False,
                    skip_group_check=True,
                )
            for k, (j, qq, c) in enumerate(mms):
                w0 = (k // WOH) * WOH
                off = k - w0
                nc.tensor.matmul(
                    ps[:, j * 128 : (j + 1) * 128],
                    lhsT=gts[qq][:, c * 128 : (c + 1) * 128],
                    rhs=ohs[w0][:, off * 128 : (off + 1) * 128],
                    start=False,
                    stop=(k == n_mm_g - 1),
                    skip_group_check=True,
                )
            return (L, g, blocks, ps)

        def epi_phase(st):
            L, g, blocks, ps = st
            nj = len(blocks)
            w = nj * 128
            gb = g * G * 128
            ob = outsb.tile([128, G * 128], F32, name="ob", tag="ob")
            nc.vector.tensor_tensor(
                ob[:, :w], ps[:, :w], dd_sb[:, gb : gb + w],
                mybir.AluOpType.mult,
            )
            obf = obbf.tile([128, G * 128], BF16, name="obf", tag="obf")
            nc.scalar.activation(
                obf[:, :w], ob[:, :w], ID.Identity, bias=bias(L)
            )
            nc.sync.dma_start(
                out=h_out[:, L * s_pad + gb : L * s_pad + gb + w],
                in_=obf[:, :w],
            )
            if L < 2:
                r = rsb.tile([128, G * 128], BF16, name="r", tag="r")
                nc.scalar.activation(r[:, :w], ob[:, :w], ID.Relu, bias=bias(L))
                ps2 = mm_ps.tile([128, G * 128], F32, name="ps2", tag="ps2")
                for j in range(nj):
                    nc.tensor.matmul(
                        ps2[:, j * 128 : (j + 1) * 128],
                        lhsT=r[:, j * 128 : (j + 1) * 128],
                        rhs=w_bf[:, (L + 1) * 128 : (L + 2) * 128],
                        start=True,
                        stop=True,
                        skip_group_check=True,
                    )
                for j, b in enumerate(blocks):
                    scale_table_block(b, ps2[:, j * 128 : (j + 1) * 128])
                flush_table_blocks(blocks[0], blocks[-1] + 1)
                for q in range(NQ - 1):
                    # early triggers for q0..q2; q3 is emitted lazily at the
                    # next layer's first gather so it doesn't block the
                    # gpsimd queue while the pipeline drains
                    if g == min(ag_ready_grp[q] + AG_LAG, ngrp - 1):
                        trigger_ag(L + 1, q)

        DEFER = 2  # groups whose q0-q2 gathers are emitted before the first
        # q3 gather, so AG_q3 (triggered last, at the previous layer's tail)
        # overlaps with useful generation + drain instead of stalling gpsimd
        for L in range(3):
            pending = None
            pre = {}
            for g in range(min(DEFER, ngrp)):
                for q in range(NQ - 1):
                    pre[(g, q)] = gather_call(L, g, q)
            for g in range(ngrp):
                st = agg_phase(L, g, pre)
                if pending is not None:
                    epi_phase(pending)
                pending = st
            epi_phase(pending)

    nc.compile()
    return nc


_BUILD_CACHE = {}


def _get_kernel(sched):
    key = (
        sched["nblk"], sched["s_pad"],
        tuple(tuple(c) for c in sched["caps"]),
    )
    if key not in _BUILD_CACHE:
        _BUILD_CACHE[key] = build_kernel(sched)
    return _BUILD_CACHE[key]


# ----------------------------------------------------------------------------
# Entry point
# ----------------------------------------------------------------------------

def _run(x, edge_index, W0, b0, W1, b1, W2, b2, trace=False):
    n = int(np.asarray(x).shape[0])
    s_real = n // N_CORES
    in_maps, sched = _prep_inputs(
        x, edge_index, W0, b0, W1, b1, W2, b2, s_real
    )
    s_pad = sched["s_pad"]
    nc = _get_kernel(sched)
    res = bass_utils.run_bass_kernel_spmd(
        nc, in_maps, core_ids=list(range(N_CORES)), trace=trace
    )
    outs = []
    for L in range(3):
        h = np.concatenate(
            [
                np.asarray(
                    res.results[r]["h_out"][:, L * s_pad : L * s_pad + s_real],
                    dtype=np.float32,
                )
                for r in range(N_CORES)
            ],
            axis=1,
        ).T
        outs.append(h)
    full = np.stack(outs, axis=1).astype(np.float32)
    return full, res


def kernel(**inputs):
    trace = os.environ.get("TRN_KERNEL_TRACE", "") == "1"
    out, res = _run(
        np.asarray(inputs["x"]),
        np.asarray(inputs["edge_index"]),
        np.asarray(inputs["W0"]),
        np.asarray(inputs["b0"]),
        np.asarray(inputs["W1"]),
        np.asarray(inputs["b1"]),
        np.asarray(inputs["W2"]),
        np.asarray(inputs["b2"]),
        trace=trace,
    )
    if trace and res.exec_time_ns is not None:
        print(f"HW exec time: {res.exec_time_ns} ns")
        if res.instructions_and_trace:
            print(f"trace: {res.instructions_and_trace[1]}")
    return out
